# revision 32
# baseline (speedup 1.0000x reference)
"""DIEN forward-loss kernel for Trainium2, SPMD over 8 NeuronCores. V3.

Data-parallel over batch (32 rows/core), embedding replicated. Critical-path
restructure vs V2: the per-step blend h' = g*T + (1-g)*h is split into
p = g*tanh(..) and q = (1-g)*h; since W@h' = W@p + W@q, the q-side h-matmuls
for the next step issue during the current tanh window and only the p-side
matmuls remain on the serial chain. Sigmoid is split into an early r-gate
sigmoid (gates the candidate product) and a later update-gate sigmoid (only
needed post-tanh), each in its own PSUM bank so they don't falsely couple.
The x-side preactivations land in per-step PSUM regions via identity-inject
matmuls from chunk-staged SBUF buffers; candidate x-terms are staged
interleaved ([xn_G | xh_A] per step) so t2 assembly is two DVE ops. GRU and
AUGRU run LAG=8 apart (one chunk) with AUGRU x-chunks emitted in half-chunks
to meet the tighter deadline. Aux-gram and AUGRU x-matmuls read the hidden
ring directly (strided rhs), no contiguous copy. BCE uses Softplus.

PSUM banks: pgr(x2) r-gate preacts [rG|rA], pgg(x2) update-gate preacts
[gG|uA], pn(x2) candidate h-side [nhG+bhhn|nhA], ck(x2) chunk-stage/
transposes/gram/MLP.
"""
import numpy as np
import concourse.bass as bass
import concourse.bacc as bacc
import concourse.mybir as mybir
import concourse.tile as tile
from concourse.tile import add_dep_helper
from concourse.bass_utils import run_bass_kernel_spmd
from concourse.masks import make_identity

F32 = mybir.dt.float32
BF16 = mybir.dt.bfloat16
I32 = mybir.dt.int32
AF = mybir.ActivationFunctionType
OP = mybir.AluOpType

B, L, D, NV = 256, 200, 128, 500000
NCORES = 8
BL = B // NCORES          # 32 batch rows per core
NT = L * BL               # 6400 (t,b) pairs per core
NTIL = NT // 128          # 50 gather tiles
CH = 8                    # timesteps per chunk
CW = CH * BL              # 256 cols per chunk gate region
NCHUNK = L // CH          # 25
LAG = CH                  # AUGRU lags GRU by one chunk
NSLOT = L + LAG           # 208
EPS_BN = 1e-5
DICE_A = 0.1
ALPHA = 0.2
MAGIC = 0x5F3759DF
PW = BL + 4               # 36: [hA(32) | aux | pad]
DEPRI = 10 ** 6           # scheduler de-priority offset for off-chain work


def _rsqrt(nc, pool, v, out, shape, iters=3):
    """out = 1/sqrt(v) on DVE (quake seed + Newton). v >= 0."""
    p, n = shape
    iv = out.bitcast(I32)
    nc.vector.tensor_scalar(out=iv, in0=v.bitcast(I32), scalar1=1,
                            scalar2=None, op0=OP.arith_shift_right)
    nc.vector.tensor_scalar(out=iv, in0=iv, scalar1=-1, scalar2=None,
                            op0=OP.bitwise_xor)
    nc.vector.tensor_scalar(out=iv, in0=iv, scalar1=MAGIC + 1, scalar2=None,
                            op0=OP.add)
    t = pool.tile([p, n], F32, tag="rsqrt_t")
    for _ in range(iters):
        nc.vector.tensor_tensor(out=t[:], in0=v, in1=out, op=OP.mult)
        nc.vector.tensor_tensor(out=t[:], in0=t[:], in1=out, op=OP.mult)
        nc.vector.tensor_scalar(out=t[:], in0=t[:], scalar1=-0.5, scalar2=1.5,
                                op0=OP.mult, op1=OP.add)
        nc.vector.tensor_tensor(out=out, in0=out, in1=t[:], op=OP.mult)


def build_bass(upto="full"):
    nc = bacc.Bacc("TRN2", target_bir_lowering=False, num_devices=NCORES)

    # ---------------- kernel parameters ----------------
    emb = nc.declare_dram_parameter("emb", [NV, D], F32, isOutput=False)
    idx_h = nc.declare_dram_parameter("idx_h", [128, NTIL], I32, isOutput=False)
    y_h = nc.declare_dram_parameter("y_h", [128, NTIL], F32, isOutput=False)
    idx_t = nc.declare_dram_parameter("idx_t", [B, 1], I32, isOutput=False)
    # GRU weights (bf16): x-side / h-side, gate order [r | zbar]
    wgx = nc.declare_dram_parameter("wgx", [D, 2 * D], BF16, isOutput=False)
    wgh = nc.declare_dram_parameter("wgh", [D, 2 * D], BF16, isOutput=False)
    wnx = nc.declare_dram_parameter("wnx", [D, D], BF16, isOutput=False)
    wnh = nc.declare_dram_parameter("wnh", [D, D], BF16, isOutput=False)
    bg_row = nc.declare_dram_parameter("bg_row", [1, 2 * D], BF16, isOutput=False)
    bihn_r = nc.declare_dram_parameter("bihn_r", [1, D], BF16, isOutput=False)
    bhhn_r = nc.declare_dram_parameter("bhhn_r", [1, D], BF16, isOutput=False)
    # AUGRU weights (bf16): gate order [r | u]
    aux_w = nc.declare_dram_parameter("aux_w", [D, 2 * D], BF16, isOutput=False)
    auh_w = nc.declare_dram_parameter("auh_w", [D, 2 * D], BF16, isOutput=False)
    ahx_w = nc.declare_dram_parameter("ahx_w", [D, D], BF16, isOutput=False)
    ahh_w = nc.declare_dram_parameter("ahh_w", [D, D], BF16, isOutput=False)
    ba_row = nc.declare_dram_parameter("ba_row", [1, 2 * D], BF16, isOutput=False)
    bh_r = nc.declare_dram_parameter("bh_r", [1, D], BF16, isOutput=False)
    # final MLP (bf16)
    W1a = nc.declare_dram_parameter("W1a", [D, D], BF16, isOutput=False)
    W1b = nc.declare_dram_parameter("W1b", [D, D], BF16, isOutput=False)
    b1 = nc.declare_dram_parameter("b1", [1, D], BF16, isOutput=False)
    W2 = nc.declare_dram_parameter("W2", [D, D // 2], BF16, isOutput=False)
    b2 = nc.declare_dram_parameter("b2", [1, D // 2], BF16, isOutput=False)
    Wf = nc.declare_dram_parameter("Wf", [D // 2, 1], BF16, isOutput=False)
    bf = nc.declare_dram_parameter("bf", [1, 1], BF16, isOutput=False)
    h0T = nc.declare_dram_parameter("h0T", [D, BL], BF16, isOutput=False)
    y_t = nc.declare_dram_parameter("y_t", [1, B], F32, isOutput=False)
    out_p = nc.declare_dram_parameter("out", [1, 1], F32, isOutput=True)

    ploc = nc.dram_tensor("ploc", [D, PW], BF16)
    gall = nc.dram_tensor("gall", [NCORES * D, PW], BF16)

    with tile.TileContext(nc) as tc:
        with (
            tc.tile_pool(name="persist", bufs=1) as pp,
            tc.tile_pool(name="work", bufs=3) as wk,
            tc.tile_pool(name="pq", bufs=3) as pqp,
            tc.tile_pool(name="ps_pgr", bufs=2, space="PSUM") as ppgr,
            tc.tile_pool(name="ps_pgg", bufs=2, space="PSUM") as ppgg,
            tc.tile_pool(name="ps_pn", bufs=2, space="PSUM") as ppn,
            tc.tile_pool(name="ps_ck", bufs=2, space="PSUM") as pck,
        ):
            # ---------------- index loads FIRST (gathers gate the head) ----
            idx_s = pp.tile([128, NTIL], I32, tag="idx_s")
            nc.sync.dma_start(out=idx_s[:], in_=idx_h[:])
            idx_t_s = pp.tile([128, 2], I32, tag="idx_t_s")
            idx_t_d = idx_t[:].rearrange("(k p) w -> p (k w)", k=2)
            nc.sync.dma_start(out=idx_t_s[:], in_=idx_t_d)

            # constants on the gpsimd queue BEFORE the gather stream
            identf = pp.tile([128, 128], F32, tag="identf")
            make_identity(nc, identf[:])
            identb = pp.tile([128, 128], BF16, tag="identb")
            nc.vector.tensor_copy(identb[:], identf[:])
            ones_row = pp.tile([1, CW], BF16, tag="ones_row")
            nc.gpsimd.memset(ones_row[:], 1.0)
            ones_b = pp.tile([1, B], BF16, tag="ones_b")
            nc.gpsimd.memset(ones_b[:], 1.0)
            ones_col = pp.tile([128, 1], F32, tag="ones_col")
            nc.gpsimd.memset(ones_col[:], 1.0)

            X = pp.tile([128, (NSLOT + 1) * 2 * BL], BF16, tag="X")
            X_v = X[:].rearrange("p (s w) -> p s w", w=2 * BL)
            nc.gpsimd.memset(X[:, 0:2 * BL], 0.0)

            erows = pp.tile([128, NT], F32, tag="erows")
            itemr = pp.tile([128, 2 * D], F32, tag="itemr")
            for k in range(NTIL):
                nc.gpsimd.indirect_dma_start(
                    out=erows[:, 128 * k:128 * (k + 1)], out_offset=None,
                    in_=emb[:],
                    in_offset=bass.IndirectOffsetOnAxis(ap=idx_s[:, k:k + 1],
                                                        axis=0),
                )
            # all 256 target items gathered locally (replicated)
            for k in range(2):
                nc.gpsimd.indirect_dma_start(
                    out=itemr[:, k * D:(k + 1) * D], out_offset=None,
                    in_=emb[:],
                    in_offset=bass.IndirectOffsetOnAxis(
                        ap=idx_t_s[:, k:k + 1], axis=0),
                )

            def load(ap, shape, tag, dt=BF16):
                t = pp.tile(shape, dt, tag=tag)
                nc.sync.dma_start(out=t[:], in_=ap[:])
                return t

            wgx_s = load(wgx, [D, 2 * D], "wgx")
            wgh_s = load(wgh, [D, 2 * D], "wgh")
            wnx_s = load(wnx, [D, D], "wnx")
            wnh_s = load(wnh, [D, D], "wnh")
            bg_s = load(bg_row, [1, 2 * D], "bg")
            bihn_s = load(bihn_r, [1, D], "bihn")
            bhhn_s = load(bhhn_r, [1, D], "bhhn")
            aux_s = load(aux_w, [D, 2 * D], "aux")
            auh_s = load(auh_w, [D, 2 * D], "auh")
            ahx_s = load(ahx_w, [D, D], "ahx")
            ahh_s = load(ahh_w, [D, D], "ahh")
            ba_s = load(ba_row, [1, 2 * D], "ba")
            bh_s = load(bh_r, [1, D], "bh")
            W1a_s = load(W1a, [D, D], "W1a")
            W1b_s = load(W1b, [D, D], "W1b")
            b1_s = load(b1, [1, D], "b1")
            W2_s = load(W2, [D, D // 2], "W2")
            b2_s = load(b2, [1, D // 2], "b2")
            Wf_s = load(Wf, [D // 2, 1], "Wf")
            bf_s = load(bf, [1, 1], "bf")
            y_t_s = load(y_t, [1, B], "y_t", F32)
            y_h_s = load(y_h, [128, NTIL], "y_h", F32)
            h0s = load(h0T, [D, BL], "h0s")

            # persistent big buffers
            ET = pp.tile([128, NT], BF16, tag="ET")
            ss_all = pp.tile([128, NTIL], F32, tag="ss_all")
            sc_all = pp.tile([128, NTIL], F32, tag="sc_all")
            s_all = pp.tile([128, NTIL], F32, tag="s_all")
            # chunk staging buffers (manual double-buffer)
            gx0 = pp.tile([128, 2 * CW], BF16, tag="gx0")
            gx1 = pp.tile([128, 2 * CW], BF16, tag="gx1")
            ax0 = pp.tile([128, 2 * CW], BF16, tag="ax0")
            ax1 = pp.tile([128, 2 * CW], BF16, tag="ax1")
            xc0 = pp.tile([128, 2 * CW], BF16, tag="xc0")
            xc1 = pp.tile([128, 2 * CW], BF16, tag="xc1")
            gx, ax, xc = [gx0, gx1], [ax0, ax1], [xc0, xc1]

            # hA(-1) = h0, read by slot LAG
            nc.vector.tensor_copy(X_v[:, LAG, BL:2 * BL], h0s[:])

            # ---------- phase A: norm + transpose pipeline ----------
            def do_tile(k):
                er = erows[:, 128 * k:128 * (k + 1)]
                sq = wk.tile([128, 128], F32, tag="sq_scr")
                nc.vector.scalar_tensor_tensor(
                    out=sq[:], in0=er, scalar=0.0, in1=er,
                    op0=OP.add, op1=OP.mult, accum_out=ss_all[:, k:k + 1])
                if k % 4 == 3 or k == NTIL - 1:
                    k0 = (k // 4) * 4
                    w = k - k0 + 1
                    _rsqrt(nc, wk, ss_all[:, k0:k + 1], sc_all[:, k0:k + 1],
                           [128, w], iters=1)
                    nc.vector.tensor_scalar_min(out=sc_all[:, k0:k + 1],
                                                in0=sc_all[:, k0:k + 1],
                                                scalar1=1.0)

            def do_tile2(j):
                erj = erows[:, 128 * j:128 * (j + 1)]
                ersc = wk.tile([128, 128], BF16, tag="ersc")
                nc.vector.tensor_scalar(out=ersc[:], in0=erj,
                                        scalar1=sc_all[:, j:j + 1],
                                        scalar2=None, op0=OP.mult)
                tp = pck.tile([128, 512], F32, tag="ck")
                tpb = tp[:].bitcast(BF16)
                nc.tensor.transpose(out=tpb[:, 0:128], in_=ersc[:],
                                    identity=identb[:])
                nc.vector.tensor_copy(ET[:, 128 * j:128 * (j + 1)],
                                      tpb[:, 0:128])

            # ---------- chunk emissions ----------
            def emit_gx(c):
                # GRU x-gate preacts for chunk c -> gx[c%2]: [rx(256)|zbx(256)]
                ecols = ET[:, c * CW:(c + 1) * CW]
                ck = pck.tile([128, 512], F32, tag="ck")
                nc.tensor.matmul(ck[:, 0:CW], wgx_s[:, 0:D], ecols,
                                 start=True, stop=False)
                nc.tensor.matmul(ck[:, CW:2 * CW], wgx_s[:, D:2 * D], ecols,
                                 start=True, stop=False)
                nc.tensor.matmul(ck[:, 0:CW], bg_s[0:1, 0:D],
                                 ones_row[0:1, :], start=False, stop=True)
                nc.tensor.matmul(ck[:, CW:2 * CW], bg_s[0:1, D:2 * D],
                                 ones_row[0:1, :], start=False, stop=True)
                nc.vector.tensor_copy(gx[c % 2][:, 0:CW], ck[:, 0:CW])
                nc.vector.tensor_copy(gx[c % 2][:, CW:2 * CW], ck[:, CW:2 * CW])

            def emit_xn(c):
                # GRU candidate x-preact chunk c -> xc[c%2] interleaved slots
                ecols = ET[:, c * CW:(c + 1) * CW]
                ck = pck.tile([128, 512], F32, tag="ck")
                nc.tensor.matmul(ck[:, 0:CW], wnx_s[:], ecols,
                                 start=True, stop=False)
                nc.tensor.matmul(ck[:, 0:CW], bihn_s[0:1, :],
                                 ones_row[0:1, :], start=False, stop=True)
                dst = xc[c % 2][:].rearrange("p (s w) -> p s w", w=2 * BL)
                src = ck[:, 0:CW].rearrange("p (s w) -> p s w", w=BL)
                nc.vector.tensor_copy(dst[:, 0:4, 0:BL], src[:, 0:4, :])
                nc.vector.tensor_copy(dst[:, 4:8, 0:BL], src[:, 4:8, :])

            def emit_ah(ca, half):
                # AUGRU x-preacts for chunk ca, steps 4*half..4*half+3.
                # Gates -> ax[(ca+1)%2] ([rx|ux]), candidate -> xc[(ca+1)%2].
                t0 = ca * CH + 4 * half
                ocols = X_v[:, t0 + 1:t0 + 5, 0:BL]      # outs t0..t0+3
                hw = 4 * BL                              # 128
                ck = pck.tile([128, 512], F32, tag="ck")
                nc.tensor.matmul(ck[:, 0:hw], aux_s[:, 0:D], ocols,
                                 start=True, stop=False)
                nc.tensor.matmul(ck[:, hw:2 * hw], aux_s[:, D:2 * D], ocols,
                                 start=True, stop=False)
                nc.tensor.matmul(ck[:, 0:hw], ba_s[0:1, 0:D],
                                 ones_row[0:1, 0:hw], start=False, stop=True)
                nc.tensor.matmul(ck[:, hw:2 * hw], ba_s[0:1, D:2 * D],
                                 ones_row[0:1, 0:hw], start=False, stop=True)
                nc.tensor.matmul(ck[:, 2 * hw:3 * hw], ahx_s[:], ocols,
                                 start=True, stop=False)
                nc.tensor.matmul(ck[:, 2 * hw:3 * hw], bh_s[0:1, :],
                                 ones_row[0:1, 0:hw], start=False, stop=True)
                axd = ax[(ca + 1) % 2][:].rearrange("p (g w) -> p g w", g=2)
                nc.vector.tensor_copy(axd[:, 0, half * hw:(half + 1) * hw],
                                      ck[:, 0:hw])
                nc.vector.tensor_copy(axd[:, 1, half * hw:(half + 1) * hw],
                                      ck[:, hw:2 * hw])
                dst = xc[(ca + 1) % 2][:].rearrange("p (s w) -> p s w",
                                                    w=2 * BL)
                srch = ck[:, 2 * hw:3 * hw].rearrange("p (s w) -> p s w", w=BL)
                nc.vector.tensor_copy(dst[:, 4 * half:4 * half + 4, BL:2 * BL],
                                      srch[:, :, :])

            iT = pp.tile([128, B], BF16, tag="iT")

            def prep_item():
                # renorm all 256 target items, transpose into iT [D, B]
                for k in range(2):
                    itk = itemr[:, k * D:(k + 1) * D]
                    sqt = wk.tile([128, D], F32, tag="sqt")
                    sst = wk.tile([128, 1], F32, tag="sst")
                    nc.vector.scalar_tensor_tensor(
                        out=sqt[:], in0=itk, scalar=0.0, in1=itk,
                        op0=OP.add, op1=OP.mult, accum_out=sst[:])
                    sct = wk.tile([128, 1], F32, tag="sct")
                    _rsqrt(nc, wk, sst[:], sct[:], [128, 1], iters=2)
                    nc.vector.tensor_scalar_min(out=sct[:], in0=sct[:],
                                                scalar1=1.0)
                    itsc = wk.tile([128, D], BF16, tag="itsc")
                    nc.vector.tensor_scalar(out=itsc[:], in0=itk,
                                            scalar1=sct[:], scalar2=None,
                                            op0=OP.mult)
                    tp = pck.tile([128, 512], F32, tag="ck")
                    tpb = tp[:].bitcast(BF16)
                    nc.tensor.transpose(out=tpb[:, 0:D], in_=itsc[:],
                                        identity=identb[:])
                    nc.vector.tensor_copy(iT[:, k * D:(k + 1) * D],
                                          tpb[:, 0:D])

            def emit_gram(blk):
                gps = pck.tile([128, 512], F32, tag="ck")
                nc.tensor.matmul(gps[:, 0:128], ET[:, 128 * blk:128 * (blk + 1)],
                                 X_v[:, 4 * blk + 1:4 * blk + 5, 0:BL],
                                 start=True, stop=True)
                gsc = wk.tile([128, 128], F32, tag="gram_scr")
                nc.vector.scalar_tensor_tensor(
                    out=gsc[:], in0=gps[:, 0:128], scalar=1.0, in1=identf[:],
                    op0=OP.mult, op1=OP.mult,
                    accum_out=s_all[:, blk:blk + 1])

            # ---------- upfront pipeline fill ----------
            for k in range(8):
                do_tile(k)
            for j in range(6):
                do_tile2(j)
            emit_gx(0)
            emit_xn(0)

            if upto == "A":
                for k in range(8, NTIL):
                    do_tile(k)
                for j in range(6, NTIL):
                    do_tile2(j)
                dbg = wk.tile([1, 1], F32, tag="res")
                nc.vector.reduce_sum(out=dbg[:], in_=ET[0:1, 0:128],
                                     axis=mybir.AxisListType.X)
                nc.sync.dma_start(out=out_p[:], in_=dbg[:])

            # ---------- recurrence slot loop ----------
            # pg layouts: pgr = [rG|rA], pgg = [gG|uA]; gp = [rG rA gG uA]
            prev_p = prev_q = None
            if upto != "A":
                for s in range(NSLOT):
                    tg, ta, o, c = s, s - LAG, s % CH, s // CH
                    gru = tg < L
                    aug = 0 <= ta < L
                    if gru and aug:
                        cs = slice(0, 2 * BL)       # pair columns
                    elif gru:
                        cs = slice(0, BL)
                    else:
                        cs = slice(BL, 2 * BL)
                    gxc, axc, xcc = gx[c % 2], ax[c % 2], xc[c % 2]

                    # ---- per-step PSUM bank fill ----
                    pgr = ppgr.tile([128, 2 * BL], F32, tag="pgr")
                    pgg = ppgg.tile([128, 2 * BL], F32, tag="pgg")
                    pn = ppn.tile([128, 2 * BL], F32, tag="pn")
                    if gru:
                        gxv = gxc[:].rearrange("p (g w) -> p g w", g=2)
                        # injects: r-gate then update-gate x-preacts
                        nc.tensor.matmul(pgr[:, 0:BL], identb[:],
                                         gxv[:, 0, o * BL:(o + 1) * BL],
                                         start=True, stop=(s == 0))
                        nc.tensor.matmul(pgg[:, 0:BL], identb[:],
                                         gxv[:, 1, o * BL:(o + 1) * BL],
                                         start=True, stop=(s == 0))
                        nc.tensor.matmul(pn[:, 0:BL], bhhn_s[0:1, :],
                                         ones_row[0:1, 0:BL],
                                         start=True, stop=(s == 0))
                    if aug:
                        axv = axc[:].rearrange("p (g w) -> p g w", g=2)
                        nc.tensor.matmul(pgr[:, BL:2 * BL], identb[:],
                                         axv[:, 0, o * BL:(o + 1) * BL],
                                         start=True, stop=False)
                        nc.tensor.matmul(pgg[:, BL:2 * BL], identb[:],
                                         axv[:, 1, o * BL:(o + 1) * BL],
                                         start=True, stop=False)
                    if s == LAG:
                        # AUGRU step 0: h-side terms come straight from h0
                        nc.tensor.matmul(pgr[:, BL:2 * BL], auh_s[:, 0:D],
                                         h0s[:], start=False, stop=True)
                        nc.tensor.matmul(pgg[:, BL:2 * BL], auh_s[:, D:2 * D],
                                         h0s[:], start=False, stop=True)
                        nc.tensor.matmul(pn[:, BL:2 * BL], ahh_s[:], h0s[:],
                                         start=True, stop=True)
                    # q-side h-matmuls (prev_q ready during prev tanh window)
                    if s > 0 and gru:
                        qG = prev_q[:, 0:BL]
                        nc.tensor.matmul(pgr[:, 0:BL], wgh_s[:, 0:D], qG,
                                         start=False, stop=False)
                        nc.tensor.matmul(pgg[:, 0:BL], wgh_s[:, D:2 * D], qG,
                                         start=False, stop=False)
                        nc.tensor.matmul(pn[:, 0:BL], wnh_s[:], qG,
                                         start=False, stop=False)
                    if aug and ta > 0:
                        qA = prev_q[:, BL:2 * BL]
                        nc.tensor.matmul(pgr[:, BL:2 * BL], auh_s[:, 0:D], qA,
                                         start=False, stop=False)
                        nc.tensor.matmul(pgg[:, BL:2 * BL], auh_s[:, D:2 * D],
                                         qA, start=False, stop=False)
                        nc.tensor.matmul(pn[:, BL:2 * BL], ahh_s[:], qA,
                                         start=True, stop=False)

                    # ---- off-chain work: de-prioritized, fills idle ----
                    with tc.high_priority(offset=-DEPRI):
                        if o == 1 and c + 1 < NCHUNK:
                            emit_gx(c + 1)
                        if o == 3 and c + 1 < NCHUNK:
                            emit_xn(c + 1)
                        if o == 4 and c < NCHUNK:
                            emit_ah(c, 0)
                        if o == 0 and 1 <= c <= NCHUNK:
                            emit_ah(c - 1, 1)
                        if s >= 2 and (s - 2) % 4 == 3 and (s - 2) // 4 < NTIL \
                                and s - 2 < L:
                            emit_gram((s - 2) // 4)
                        if s % 2 == 1 and 8 + (s - 1) // 2 < NTIL:
                            do_tile(8 + (s - 1) // 2)
                        if s % 2 == 1 and s >= 5 and 6 + (s - 5) // 2 < NTIL:
                            do_tile2(6 + (s - 5) // 2)
                        if s == 120:
                            prep_item()

                    # p-side h-matmuls: r-gates first (they gate sigma_r)
                    if s > 0 and gru:
                        pG = prev_p[:, 0:BL]
                        nc.tensor.matmul(pgr[:, 0:BL], wgh_s[:, 0:D], pG,
                                         start=False, stop=True)
                    if aug and ta > 0:
                        pA = prev_p[:, BL:2 * BL]
                        nc.tensor.matmul(pgr[:, BL:2 * BL], auh_s[:, 0:D], pA,
                                         start=False, stop=True)
                    if s > 0 and gru:
                        nc.tensor.matmul(pn[:, 0:BL], wnh_s[:], pG,
                                         start=False, stop=True)
                    if aug and ta > 0:
                        nc.tensor.matmul(pn[:, BL:2 * BL], ahh_s[:], pA,
                                         start=False, stop=True)
                    if s > 0 and gru:
                        nc.tensor.matmul(pgg[:, 0:BL], wgh_s[:, D:2 * D], pG,
                                         start=False, stop=True)
                    if aug and ta > 0:
                        nc.tensor.matmul(pgg[:, BL:2 * BL], auh_s[:, D:2 * D],
                                         pA, start=False, stop=True)

                    # ---- serial chain ----
                    gp = wk.tile([128, 4 * BL], BF16, tag="gp")
                    nc.scalar.activation(gp[:, cs], pgr[:, cs], AF.Sigmoid)
                    tprod = wk.tile([128, 2 * BL], BF16, tag="tprod")
                    nc.vector.tensor_tensor(out=tprod[:, cs], in0=pn[:, cs],
                                            in1=gp[:, cs], op=OP.mult)
                    t2 = wk.tile([128, 2 * BL], BF16, tag="t2")
                    t2i = nc.vector.tensor_tensor(
                        out=t2[:, cs], in0=tprod[:, cs],
                        in1=xcc[:, o * 2 * BL + cs.start:
                                o * 2 * BL + cs.stop],
                        op=OP.add)
                    gcs = slice(2 * BL + cs.start, 2 * BL + cs.stop)
                    nc.scalar.activation(gp[:, gcs], pgg[:, cs], AF.Sigmoid)
                    gT = wk.tile([128, 2 * BL], BF16, tag="gT")
                    nc.scalar.activation(gT[:, cs], t2[:, cs], AF.Tanh)
                    # during-tanh: zp = 1-g ; q = zp * h_prev
                    zp = wk.tile([128, 2 * BL], BF16, tag="zp")
                    zpi = nc.vector.tensor_scalar(out=zp[:, cs], in0=gp[:, gcs],
                                                  scalar1=-1.0, scalar2=1.0,
                                                  op0=OP.mult, op1=OP.add)
                    add_dep_helper(zpi.ins, t2i.ins, sync=False,
                                   reason="keep zp off the t2 chain")
                    q = pqp.tile([128, 2 * BL], BF16, tag="q")
                    nc.vector.tensor_tensor(out=q[:, cs], in0=zp[:, cs],
                                            in1=X_v[:, s, cs], op=OP.mult)
                    # post-tanh: p = g*T ; h' = p + q
                    p = pqp.tile([128, 2 * BL], BF16, tag="p")
                    nc.vector.tensor_tensor(out=p[:, cs], in0=gp[:, gcs],
                                            in1=gT[:, cs], op=OP.mult)
                    nc.vector.tensor_tensor(out=X_v[:, s + 1, cs],
                                            in0=p[:, cs], in1=q[:, cs],
                                            op=OP.add)
                    prev_p, prev_q = p, q

            if upto == "G":
                dbg = wk.tile([1, 1], F32, tag="res")
                dbf = wk.tile([1, BL], F32, tag="resb")
                nc.vector.tensor_copy(dbf[:], X_v[0:1, L, 0:BL])
                nc.vector.reduce_sum(out=dbg[:], in_=dbf[:],
                                     axis=mybir.AxisListType.X)
                nc.sync.dma_start(out=out_p[:], in_=dbg[:])
            if upto == "GA":
                dbg = wk.tile([1, 1], F32, tag="res")
                dbf = wk.tile([1, BL], F32, tag="resb")
                nc.vector.tensor_copy(dbf[:], X_v[0:1, NSLOT, BL:2 * BL])
                nc.vector.reduce_sum(out=dbg[:], in_=dbf[:],
                                     axis=mybir.AxisListType.X)
                nc.sync.dma_start(out=out_p[:], in_=dbg[:])

            if upto in ("X", "full"):
                # ---------- aux BCE partial: sp(s) + y*(sp(-s)-sp(s)) ------
                ebuf = pp.tile([128, NTIL], F32, tag="ebuf")
                nc.scalar.activation(ebuf[:], s_all[:], AF.Exp)
                nc.vector.tensor_scalar_add(out=ebuf[:], in0=ebuf[:],
                                            scalar1=1.0)
                sp = pp.tile([128, NTIL], F32, tag="sp")
                nc.scalar.activation(sp[:], ebuf[:], AF.Ln)
                spm = pp.tile([128, NTIL], F32, tag="spm")
                nc.vector.tensor_tensor(out=spm[:], in0=sp[:], in1=s_all[:],
                                        op=OP.subtract)
                nc.vector.tensor_scalar_min(out=spm[:], in0=spm[:],
                                            scalar1=100.0)
                nc.vector.tensor_scalar_min(out=sp[:], in0=sp[:], scalar1=100.0)
                nc.vector.tensor_tensor(out=spm[:], in0=spm[:], in1=sp[:],
                                        op=OP.subtract)
                nc.vector.tensor_tensor(out=spm[:], in0=y_h_s[:], in1=spm[:],
                                        op=OP.mult)
                nc.vector.tensor_tensor(out=sp[:], in0=sp[:], in1=spm[:],
                                        op=OP.add)
                rsum = wk.tile([128, 1], F32, tag="rsum")
                nc.vector.reduce_sum(out=rsum[:], in_=sp[:],
                                     axis=mybir.AxisListType.X)
                aux_ps = pck.tile([128, 512], F32, tag="ck")
                nc.tensor.matmul(aux_ps[0:1, 0:1], rsum[:], ones_col[:, 0:1],
                                 start=True, stop=True)

                # ---------- pack + AllGather (bf16) ----------
                stage = pp.tile([D, PW], BF16, tag="stage")
                nc.gpsimd.memset(stage[:], 0.0)
                nc.vector.tensor_copy(stage[:, 0:BL], X_v[:, NSLOT, BL:2 * BL])
                nc.vector.tensor_copy(stage[0:1, BL:BL + 1],
                                      aux_ps[0:1, 0:1])
                nc.sync.dma_start(out=ploc[:], in_=stage[:])
                nc.gpsimd.collective_compute(
                    "AllGather", OP.bypass,
                    replica_groups=[list(range(NCORES))],
                    ins=[ploc[:]], outs=[gall[:]],
                )

            if upto == "X":
                dbg = wk.tile([1, 1], F32, tag="res")
                nc.vector.tensor_copy(dbg[:], aux_ps[0:1, 0:1])
                nc.sync.dma_start(out=out_p[:], in_=dbg[:])

            if upto == "full":
                # ---------- replicated final MLP ----------
                gat = pp.tile([D, NCORES * PW], BF16, tag="gat")
                for cc in range(NCORES):
                    nc.sync.dma_start(out=gat[:, cc * PW:(cc + 1) * PW],
                                      in_=gall[cc * D:(cc + 1) * D, :])
                gat_v = gat[:].rearrange("p (c w) -> p c w", c=NCORES)
                hT_v = gat_v[:, :, 0:BL]            # [128, 8, 32]
                aux8 = wk.tile([1, NCORES], F32, tag="aux8")
                aux8_v = aux8[:].rearrange("p (c w) -> p c w", w=1)
                nc.vector.tensor_copy(aux8_v, gat_v[0:1, :, BL:BL + 1])
                aux_tot = wk.tile([1, 1], F32, tag="aux_tot")
                nc.vector.reduce_sum(out=aux_tot[:], in_=aux8[:],
                                     axis=mybir.AxisListType.X)

                def dice(z_ps, pdim):
                    m = wk.tile([pdim, 1], F32, tag="dice_m")
                    nc.vector.reduce_sum(out=m[:], in_=z_ps[:],
                                         axis=mybir.AxisListType.X)
                    nc.vector.tensor_scalar_mul(out=m[:], in0=m[:],
                                                scalar1=1.0 / B)
                    xcen = wk.tile([pdim, B], F32, tag="dice_xc")
                    nc.vector.tensor_scalar(out=xcen[:], in0=z_ps[:],
                                            scalar1=m[:], scalar2=None,
                                            op0=OP.subtract)
                    sq2 = wk.tile([pdim, B], F32, tag="dice_sq")
                    vs = wk.tile([pdim, 1], F32, tag="dice_vs")
                    nc.scalar.activation(sq2[:], xcen[:], AF.Square,
                                         accum_out=vs[:])
                    nc.vector.tensor_scalar(out=vs[:], in0=vs[:],
                                            scalar1=1.0 / B, scalar2=EPS_BN,
                                            op0=OP.mult, op1=OP.add)
                    inv = wk.tile([pdim, 1], F32, tag="dice_inv")
                    _rsqrt(nc, wk, vs[:], inv[:], [pdim, 1], iters=2)
                    pr = wk.tile([pdim, B], F32, tag="dice_p")
                    nc.scalar.activation(pr[:], xcen[:], AF.Sigmoid,
                                         scale=inv[:, 0:1])
                    nc.vector.tensor_scalar(out=pr[:], in0=pr[:],
                                            scalar1=1 - DICE_A, scalar2=DICE_A,
                                            op0=OP.mult, op1=OP.add)
                    zd = wk.tile([pdim, B], BF16, tag="dice_zd")
                    nc.vector.tensor_tensor(out=zd[:], in0=z_ps[:], in1=pr[:],
                                            op=OP.mult)
                    return zd

                z1_ps = pck.tile([128, 512], F32, tag="ck")
                nc.tensor.matmul(z1_ps[:, 0:B], W1a_s[:], hT_v,
                                 start=True, stop=False)
                nc.tensor.matmul(z1_ps[:, 0:B], W1b_s[:], iT[:, :],
                                 start=False, stop=False)
                nc.tensor.matmul(z1_ps[:, 0:B], b1_s[0:1, :], ones_b[0:1, :],
                                 start=False, stop=True)
                z1d = dice(z1_ps[:, 0:B], 128)

                z2_ps = pck.tile([128, 512], F32, tag="ck")
                nc.tensor.matmul(z2_ps[0:D // 2, 0:B], W2_s[:, :], z1d[:],
                                 start=True, stop=False)
                nc.tensor.matmul(z2_ps[0:D // 2, 0:B], b2_s[0:1, :],
                                 ones_b[0:1, :], start=False, stop=True)
                z2d = dice(z2_ps[0:D // 2, 0:B], D // 2)

                s_ps = pck.tile([128, 512], F32, tag="ck")
                nc.tensor.matmul(s_ps[0:1, 0:B], Wf_s[:, 0:1], z2d[:],
                                 start=True, stop=False)
                nc.tensor.matmul(s_ps[0:1, 0:B], bf_s[0:1, 0:1],
                                 ones_b[0:1, :], start=False, stop=True)
                s_sb = wk.tile([1, B], F32, tag="s_sb")
                nc.vector.tensor_copy(s_sb[:], s_ps[0:1, 0:B])

                e2 = wk.tile([1, B], F32, tag="e2")
                nc.scalar.activation(e2[:], s_sb[:], AF.Exp)
                nc.vector.tensor_scalar_add(out=e2[:], in0=e2[:], scalar1=1.0)
                sp2 = wk.tile([1, B], F32, tag="sp2")
                nc.scalar.activation(sp2[:], e2[:], AF.Ln)
                spm2 = wk.tile([1, B], F32, tag="spm2")
                nc.vector.tensor_tensor(out=spm2[:], in0=sp2[:], in1=s_sb[:],
                                        op=OP.subtract)
                nc.vector.tensor_scalar_min(out=spm2[:], in0=spm2[:],
                                            scalar1=100.0)
                nc.vector.tensor_scalar_min(out=sp2[:], in0=sp2[:],
                                            scalar1=100.0)
                nc.vector.tensor_tensor(out=spm2[:], in0=spm2[:], in1=sp2[:],
                                        op=OP.subtract)
                nc.vector.tensor_tensor(out=spm2[:], in0=y_t_s[:], in1=spm2[:],
                                        op=OP.mult)
                nc.vector.tensor_tensor(out=sp2[:], in0=sp2[:], in1=spm2[:],
                                        op=OP.add)
                rec_sum = wk.tile([1, 1], F32, tag="rec_sum")
                nc.vector.reduce_sum(out=rec_sum[:], in_=sp2[:],
                                     axis=mybir.AxisListType.X)

                nc.vector.tensor_scalar_mul(out=aux_tot[:], in0=aux_tot[:],
                                            scalar1=ALPHA / (B * L))
                nc.vector.tensor_scalar_mul(out=rec_sum[:], in0=rec_sum[:],
                                            scalar1=1.0 / B)
                res = wk.tile([1, 1], F32, tag="res")
                nc.vector.tensor_tensor(out=res[:], in0=aux_tot[:],
                                        in1=rec_sum[:], op=OP.add)
                nc.sync.dma_start(out=out_p[:], in_=res[:])
    nc.compile()
    return nc


_NC_CACHE = None


def _get_nc():
    global _NC_CACHE
    if _NC_CACHE is None:
        import os
        _NC_CACHE = build_bass(os.environ.get("KERNEL_UPTO", "full"))
    return _NC_CACHE


def _prep_inputs(inputs):
    f32 = np.float32
    import ml_dtypes
    bf16 = ml_dtypes.bfloat16
    emb = np.ascontiguousarray(inputs["emb"], dtype=f32)
    seqs = np.asarray(inputs["history_seqs"])
    labs = np.asarray(inputs["history_labels"])
    tgt = np.asarray(inputs["target_item"])
    tl = np.asarray(inputs["target_label"]).astype(f32)

    w_ih = np.asarray(inputs["w_ih"], dtype=f32)   # rows: [r | z | n]
    w_hh = np.asarray(inputs["w_hh"], dtype=f32)
    b_ih = np.asarray(inputs["b_ih"], dtype=f32)
    b_hh = np.asarray(inputs["b_hh"], dtype=f32)
    # gate order in banks: [r | zbar]; zbar = negated z
    wgx_m = np.concatenate([w_ih[0:D].T, -w_ih[D:2 * D].T], axis=1)
    wgh_m = np.concatenate([w_hh[0:D].T, -w_hh[D:2 * D].T], axis=1)
    bg = np.concatenate([b_ih[0:D] + b_hh[0:D],
                         -(b_ih[D:2 * D] + b_hh[D:2 * D])]).reshape(1, 2 * D)
    wnx_m = np.ascontiguousarray(w_ih[2 * D:3 * D].T)
    wnh_m = np.ascontiguousarray(w_hh[2 * D:3 * D].T)
    bihn = b_ih[2 * D:].reshape(1, D)
    bhhn = b_hh[2 * D:].reshape(1, D)

    Wu, Wr, Wh = (np.asarray(inputs[k], dtype=f32) for k in ("Wu", "Wr", "Wh"))
    Uu, Ur, Uh = (np.asarray(inputs[k], dtype=f32) for k in ("Uu", "Ur", "Uh"))
    bu = np.asarray(inputs["bu"], dtype=f32).reshape(-1)
    br = np.asarray(inputs["br"], dtype=f32).reshape(-1)
    bh = np.asarray(inputs["bh"], dtype=f32).reshape(1, D)
    aux_wm = np.concatenate([Wr, Wu], axis=1)       # [D, 2D], r first
    auh_wm = np.concatenate([Ur, Uu], axis=1)
    ba = np.concatenate([br, bu]).reshape(1, 2 * D)

    W1 = np.ascontiguousarray(inputs["W1"], dtype=f32)
    b1 = np.asarray(inputs["b1"], dtype=f32).reshape(1, D)
    W2 = np.ascontiguousarray(inputs["W2"], dtype=f32)
    b2 = np.asarray(inputs["b2"], dtype=f32).reshape(1, D // 2)
    Wf = np.ascontiguousarray(inputs["Wf"], dtype=f32)
    bfv = np.asarray(inputs["bf"], dtype=f32).reshape(1, 1)
    h0 = np.asarray(inputs["h0"], dtype=f32)
    y_t_full = tl.reshape(1, B)

    cvt = lambda a: np.ascontiguousarray(a).astype(bf16)
    shared = dict(
        emb=emb, wgx=cvt(wgx_m), wgh=cvt(wgh_m), wnx=cvt(wnx_m),
        wnh=cvt(wnh_m), bg_row=cvt(bg), bihn_r=cvt(bihn), bhhn_r=cvt(bhhn),
        aux_w=cvt(aux_wm), auh_w=cvt(auh_wm), ahx_w=cvt(Wh), ahh_w=cvt(Uh),
        ba_row=cvt(ba), bh_r=cvt(bh),
        W1a=cvt(W1[0:D]), W1b=cvt(W1[D:2 * D]),
        b1=cvt(b1), W2=cvt(W2), b2=cvt(b2), Wf=cvt(Wf), bf=cvt(bfv),
        y_t=y_t_full, idx_t=tgt.reshape(B, 1).astype(np.int32))
    in_maps = []
    for c in range(NCORES):
        sl = slice(c * BL, (c + 1) * BL)
        idx_f = np.ascontiguousarray(seqs[sl].T).reshape(-1)
        idx_hc = np.ascontiguousarray(
            idx_f.reshape(NTIL, 128).T).astype(np.int32)
        y_f = np.ascontiguousarray(labs[sl, :, 0].T).reshape(-1).astype(f32)
        y_hc = np.ascontiguousarray(y_f.reshape(NTIL, 128).T)
        h0Tc = cvt(h0[sl].T)
        m = dict(shared)
        m.update(idx_h=idx_hc, y_h=y_hc, h0T=h0Tc)
        in_maps.append(m)
    return in_maps


def kernel(**inputs) -> np.ndarray:
    nc = _get_nc()
    in_maps = _prep_inputs(inputs)
    res = run_bass_kernel_spmd(nc, in_maps, core_ids=list(range(NCORES)))
    out = np.asarray(res.results[0]["out"], dtype=np.float32)
    return out.reshape(())


# revision 34
# speedup vs baseline: 1.5225x; 1.5225x over previous
"""DIEN forward-loss kernel for Trainium2, SPMD over 8 NeuronCores. V3.

Data-parallel over batch (32 rows/core), embedding replicated. Critical-path
restructure vs V2: the per-step blend h' = g*T + (1-g)*h is split into
p = g*tanh(..) and q = (1-g)*h; since W@h' = W@p + W@q, the q-side h-matmuls
for the next step issue during the current tanh window and only the p-side
matmuls remain on the serial chain. Sigmoid is split into an early r-gate
sigmoid (gates the candidate product) and a later update-gate sigmoid (only
needed post-tanh), each in its own PSUM bank so they don't falsely couple.
The x-side preactivations land in per-step PSUM regions via identity-inject
matmuls from chunk-staged SBUF buffers; candidate x-terms are staged
interleaved ([xn_G | xh_A] per step) so t2 assembly is two DVE ops. GRU and
AUGRU run LAG=8 apart (one chunk) with AUGRU x-chunks emitted in half-chunks
to meet the tighter deadline. Aux-gram and AUGRU x-matmuls read the hidden
ring directly (strided rhs), no contiguous copy. BCE uses Softplus.

PSUM banks: pgr(x2) r-gate preacts [rG|rA], pgg(x2) update-gate preacts
[gG|uA], pn(x2) candidate h-side [nhG+bhhn|nhA], ck(x2) chunk-stage/
transposes/gram/MLP.
"""
import numpy as np
import concourse.bass as bass
import concourse.bacc as bacc
import concourse.mybir as mybir
import concourse.tile as tile
from concourse.tile import add_dep_helper
from concourse.bass_utils import run_bass_kernel_spmd
from concourse.masks import make_identity

F32 = mybir.dt.float32
BF16 = mybir.dt.bfloat16
I32 = mybir.dt.int32
AF = mybir.ActivationFunctionType
OP = mybir.AluOpType

B, L, D, NV = 256, 200, 128, 500000
NCORES = 8
BL = B // NCORES          # 32 batch rows per core
NT = L * BL               # 6400 (t,b) pairs per core
NTIL = NT // 128          # 50 gather tiles
CH = 8                    # timesteps per chunk
CW = CH * BL              # 256 cols per chunk gate region
NCHUNK = L // CH          # 25
LAG = CH                  # AUGRU lags GRU by one chunk
NSLOT = L + LAG           # 208
EPS_BN = 1e-5
DICE_A = 0.1
ALPHA = 0.2
MAGIC = 0x5F3759DF
PW = BL + 4               # 36: [hA(32) | aux | pad]
DEPRI = 10 ** 6           # scheduler de-priority offset for off-chain work


def _rsqrt(nc, pool, v, out, shape, iters=3):
    """out = 1/sqrt(v) on DVE (quake seed + Newton). v >= 0."""
    p, n = shape
    iv = out.bitcast(I32)
    nc.vector.tensor_scalar(out=iv, in0=v.bitcast(I32), scalar1=1,
                            scalar2=None, op0=OP.arith_shift_right)
    nc.vector.tensor_scalar(out=iv, in0=iv, scalar1=-1, scalar2=None,
                            op0=OP.bitwise_xor)
    nc.vector.tensor_scalar(out=iv, in0=iv, scalar1=MAGIC + 1, scalar2=None,
                            op0=OP.add)
    t = pool.tile([p, n], F32, tag="rsqrt_t")
    for _ in range(iters):
        nc.vector.tensor_tensor(out=t[:], in0=v, in1=out, op=OP.mult)
        nc.vector.tensor_tensor(out=t[:], in0=t[:], in1=out, op=OP.mult)
        nc.vector.tensor_scalar(out=t[:], in0=t[:], scalar1=-0.5, scalar2=1.5,
                                op0=OP.mult, op1=OP.add)
        nc.vector.tensor_tensor(out=out, in0=out, in1=t[:], op=OP.mult)


def build_bass(upto="full"):
    nc = bacc.Bacc("TRN2", target_bir_lowering=False, num_devices=NCORES)

    # ---------------- kernel parameters ----------------
    emb = nc.declare_dram_parameter("emb", [NV, D], F32, isOutput=False)
    idx_h = nc.declare_dram_parameter("idx_h", [128, NTIL], I32, isOutput=False)
    y_h = nc.declare_dram_parameter("y_h", [128, NTIL], F32, isOutput=False)
    idx_t = nc.declare_dram_parameter("idx_t", [B, 1], I32, isOutput=False)
    # GRU weights (bf16): x-side / h-side, gate order [r | zbar]
    wgx = nc.declare_dram_parameter("wgx", [D, 2 * D], BF16, isOutput=False)
    wgh = nc.declare_dram_parameter("wgh", [D, 2 * D], BF16, isOutput=False)
    wnx = nc.declare_dram_parameter("wnx", [D, D], BF16, isOutput=False)
    wnh = nc.declare_dram_parameter("wnh", [D, D], BF16, isOutput=False)
    bg_row = nc.declare_dram_parameter("bg_row", [1, 2 * D], BF16, isOutput=False)
    bihn_r = nc.declare_dram_parameter("bihn_r", [1, D], BF16, isOutput=False)
    bhhn_r = nc.declare_dram_parameter("bhhn_r", [1, D], BF16, isOutput=False)
    # AUGRU weights (bf16): gate order [r | u]
    aux_w = nc.declare_dram_parameter("aux_w", [D, 2 * D], BF16, isOutput=False)
    auh_w = nc.declare_dram_parameter("auh_w", [D, 2 * D], BF16, isOutput=False)
    ahx_w = nc.declare_dram_parameter("ahx_w", [D, D], BF16, isOutput=False)
    ahh_w = nc.declare_dram_parameter("ahh_w", [D, D], BF16, isOutput=False)
    ba_row = nc.declare_dram_parameter("ba_row", [1, 2 * D], BF16, isOutput=False)
    bh_r = nc.declare_dram_parameter("bh_r", [1, D], BF16, isOutput=False)
    # final MLP (bf16)
    W1a = nc.declare_dram_parameter("W1a", [D, D], BF16, isOutput=False)
    W1b = nc.declare_dram_parameter("W1b", [D, D], BF16, isOutput=False)
    b1 = nc.declare_dram_parameter("b1", [1, D], BF16, isOutput=False)
    W2 = nc.declare_dram_parameter("W2", [D, D // 2], BF16, isOutput=False)
    b2 = nc.declare_dram_parameter("b2", [1, D // 2], BF16, isOutput=False)
    Wf = nc.declare_dram_parameter("Wf", [D // 2, 1], BF16, isOutput=False)
    bf = nc.declare_dram_parameter("bf", [1, 1], BF16, isOutput=False)
    h0T = nc.declare_dram_parameter("h0T", [D, BL], BF16, isOutput=False)
    y_t = nc.declare_dram_parameter("y_t", [1, B], F32, isOutput=False)
    out_p = nc.declare_dram_parameter("out", [1, 1], F32, isOutput=True)

    ploc = nc.dram_tensor("ploc", [D, PW], BF16)
    gall = nc.dram_tensor("gall", [NCORES * D, PW], BF16)

    with tile.TileContext(nc) as tc:
        with (
            tc.tile_pool(name="persist", bufs=1) as pp,
            tc.tile_pool(name="work", bufs=3) as wk,
            tc.tile_pool(name="pq", bufs=3) as pqp,
            tc.tile_pool(name="ps_pgr", bufs=2, space="PSUM") as ppgr,
            tc.tile_pool(name="ps_pgg", bufs=2, space="PSUM") as ppgg,
            tc.tile_pool(name="ps_pn", bufs=2, space="PSUM") as ppn,
            tc.tile_pool(name="ps_ck", bufs=2, space="PSUM") as pck,
        ):
            # ---------------- index loads FIRST (gathers gate the head) ----
            idx_s = pp.tile([128, NTIL], I32, tag="idx_s")
            nc.sync.dma_start(out=idx_s[:], in_=idx_h[:])
            idx_t_s = pp.tile([128, 2], I32, tag="idx_t_s")
            idx_t_d = idx_t[:].rearrange("(k p) w -> p (k w)", k=2)
            nc.sync.dma_start(out=idx_t_s[:], in_=idx_t_d)

            # constants on the gpsimd queue BEFORE the gather stream
            identf = pp.tile([128, 128], F32, tag="identf")
            make_identity(nc, identf[:])
            identb = pp.tile([128, 128], BF16, tag="identb")
            nc.vector.tensor_copy(identb[:], identf[:])
            ones_row = pp.tile([1, CW], BF16, tag="ones_row")
            nc.gpsimd.memset(ones_row[:], 1.0)
            ones_b = pp.tile([1, B], BF16, tag="ones_b")
            nc.gpsimd.memset(ones_b[:], 1.0)
            ones_col = pp.tile([128, 1], F32, tag="ones_col")
            nc.gpsimd.memset(ones_col[:], 1.0)

            X = pp.tile([128, (NSLOT + 1) * 2 * BL], BF16, tag="X")
            X_v = X[:].rearrange("p (s w) -> p s w", w=2 * BL)
            nc.gpsimd.memset(X[:, 0:2 * BL], 0.0)

            erows = pp.tile([128, NT], F32, tag="erows")
            itemr = pp.tile([128, 2 * D], F32, tag="itemr")
            for k in range(NTIL):
                nc.gpsimd.indirect_dma_start(
                    out=erows[:, 128 * k:128 * (k + 1)], out_offset=None,
                    in_=emb[:],
                    in_offset=bass.IndirectOffsetOnAxis(ap=idx_s[:, k:k + 1],
                                                        axis=0),
                )
            # all 256 target items gathered locally (replicated)
            for k in range(2):
                nc.gpsimd.indirect_dma_start(
                    out=itemr[:, k * D:(k + 1) * D], out_offset=None,
                    in_=emb[:],
                    in_offset=bass.IndirectOffsetOnAxis(
                        ap=idx_t_s[:, k:k + 1], axis=0),
                )

            def load(ap, shape, tag, dt=BF16):
                t = pp.tile(shape, dt, tag=tag)
                nc.sync.dma_start(out=t[:], in_=ap[:])
                return t

            wgx_s = load(wgx, [D, 2 * D], "wgx")
            wgh_s = load(wgh, [D, 2 * D], "wgh")
            wnx_s = load(wnx, [D, D], "wnx")
            wnh_s = load(wnh, [D, D], "wnh")
            bg_s = load(bg_row, [1, 2 * D], "bg")
            bihn_s = load(bihn_r, [1, D], "bihn")
            bhhn_s = load(bhhn_r, [1, D], "bhhn")
            aux_s = load(aux_w, [D, 2 * D], "aux")
            auh_s = load(auh_w, [D, 2 * D], "auh")
            ahx_s = load(ahx_w, [D, D], "ahx")
            ahh_s = load(ahh_w, [D, D], "ahh")
            ba_s = load(ba_row, [1, 2 * D], "ba")
            bh_s = load(bh_r, [1, D], "bh")
            W1a_s = load(W1a, [D, D], "W1a")
            W1b_s = load(W1b, [D, D], "W1b")
            b1_s = load(b1, [1, D], "b1")
            W2_s = load(W2, [D, D // 2], "W2")
            b2_s = load(b2, [1, D // 2], "b2")
            Wf_s = load(Wf, [D // 2, 1], "Wf")
            bf_s = load(bf, [1, 1], "bf")
            y_t_s = load(y_t, [1, B], "y_t", F32)
            y_h_s = load(y_h, [128, NTIL], "y_h", F32)
            h0s = load(h0T, [D, BL], "h0s")

            # persistent big buffers
            ET = pp.tile([128, NT], BF16, tag="ET")
            ss_all = pp.tile([128, NTIL], F32, tag="ss_all")
            sc_all = pp.tile([128, NTIL], F32, tag="sc_all")
            s_all = pp.tile([128, NTIL], F32, tag="s_all")
            # chunk staging buffers (manual double-buffer)
            gx0 = pp.tile([128, 2 * CW], BF16, tag="gx0")
            gx1 = pp.tile([128, 2 * CW], BF16, tag="gx1")
            ax0 = pp.tile([128, 2 * CW], BF16, tag="ax0")
            ax1 = pp.tile([128, 2 * CW], BF16, tag="ax1")
            xc0 = pp.tile([128, 2 * CW], BF16, tag="xc0")
            xc1 = pp.tile([128, 2 * CW], BF16, tag="xc1")
            gx, ax, xc = [gx0, gx1], [ax0, ax1], [xc0, xc1]

            # hA(-1) = h0, read by slot LAG
            nc.vector.tensor_copy(X_v[:, LAG, BL:2 * BL], h0s[:])

            # ---------- phase A: norm + transpose pipeline ----------
            def do_tile(k):
                er = erows[:, 128 * k:128 * (k + 1)]
                sq = wk.tile([128, 128], F32, tag="sq_scr")
                nc.vector.scalar_tensor_tensor(
                    out=sq[:], in0=er, scalar=0.0, in1=er,
                    op0=OP.add, op1=OP.mult, accum_out=ss_all[:, k:k + 1])
                if k % 4 == 3 or k == NTIL - 1:
                    k0 = (k // 4) * 4
                    w = k - k0 + 1
                    _rsqrt(nc, wk, ss_all[:, k0:k + 1], sc_all[:, k0:k + 1],
                           [128, w], iters=1)
                    nc.vector.tensor_scalar_min(out=sc_all[:, k0:k + 1],
                                                in0=sc_all[:, k0:k + 1],
                                                scalar1=1.0)

            def do_tile2(j):
                erj = erows[:, 128 * j:128 * (j + 1)]
                ersc = wk.tile([128, 128], BF16, tag="ersc")
                nc.vector.tensor_scalar(out=ersc[:], in0=erj,
                                        scalar1=sc_all[:, j:j + 1],
                                        scalar2=None, op0=OP.mult)
                tp = pck.tile([128, 512], F32, tag="ck")
                tpb = tp[:].bitcast(BF16)
                nc.tensor.transpose(out=tpb[:, 0:128], in_=ersc[:],
                                    identity=identb[:])
                nc.vector.tensor_copy(ET[:, 128 * j:128 * (j + 1)],
                                      tpb[:, 0:128])

            # ---------- chunk emissions ----------
            def emit_gx(c):
                # GRU x-gate preacts for chunk c -> gx[c%2]: [rx(256)|zbx(256)]
                ecols = ET[:, c * CW:(c + 1) * CW]
                ck = pck.tile([128, 512], F32, tag="ck")
                nc.tensor.matmul(ck[:, 0:CW], wgx_s[:, 0:D], ecols,
                                 start=True, stop=False)
                nc.tensor.matmul(ck[:, CW:2 * CW], wgx_s[:, D:2 * D], ecols,
                                 start=True, stop=False)
                nc.tensor.matmul(ck[:, 0:CW], bg_s[0:1, 0:D],
                                 ones_row[0:1, :], start=False, stop=True)
                nc.tensor.matmul(ck[:, CW:2 * CW], bg_s[0:1, D:2 * D],
                                 ones_row[0:1, :], start=False, stop=True)
                nc.vector.tensor_copy(gx[c % 2][:, 0:CW], ck[:, 0:CW])
                nc.vector.tensor_copy(gx[c % 2][:, CW:2 * CW], ck[:, CW:2 * CW])

            def emit_xn(c):
                # GRU candidate x-preact chunk c -> xc[c%2] interleaved slots
                ecols = ET[:, c * CW:(c + 1) * CW]
                ck = pck.tile([128, 512], F32, tag="ck")
                nc.tensor.matmul(ck[:, 0:CW], wnx_s[:], ecols,
                                 start=True, stop=False)
                nc.tensor.matmul(ck[:, 0:CW], bihn_s[0:1, :],
                                 ones_row[0:1, :], start=False, stop=True)
                dst = xc[c % 2][:].rearrange("p (s w) -> p s w", w=2 * BL)
                src = ck[:, 0:CW].rearrange("p (s w) -> p s w", w=BL)
                nc.vector.tensor_copy(dst[:, 0:4, 0:BL], src[:, 0:4, :])
                nc.vector.tensor_copy(dst[:, 4:8, 0:BL], src[:, 4:8, :])

            def emit_ah(ca, half):
                # AUGRU x-preacts for chunk ca, steps 4*half..4*half+3.
                # Gates -> ax[(ca+1)%2] ([rx|ux]), candidate -> xc[(ca+1)%2].
                t0 = ca * CH + 4 * half
                ocols = X_v[:, t0 + 1:t0 + 5, 0:BL]      # outs t0..t0+3
                hw = 4 * BL                              # 128
                ck = pck.tile([128, 512], F32, tag="ck")
                nc.tensor.matmul(ck[:, 0:hw], aux_s[:, 0:D], ocols,
                                 start=True, stop=False)
                nc.tensor.matmul(ck[:, hw:2 * hw], aux_s[:, D:2 * D], ocols,
                                 start=True, stop=False)
                nc.tensor.matmul(ck[:, 0:hw], ba_s[0:1, 0:D],
                                 ones_row[0:1, 0:hw], start=False, stop=True)
                nc.tensor.matmul(ck[:, hw:2 * hw], ba_s[0:1, D:2 * D],
                                 ones_row[0:1, 0:hw], start=False, stop=True)
                nc.tensor.matmul(ck[:, 2 * hw:3 * hw], ahx_s[:], ocols,
                                 start=True, stop=False)
                nc.tensor.matmul(ck[:, 2 * hw:3 * hw], bh_s[0:1, :],
                                 ones_row[0:1, 0:hw], start=False, stop=True)
                axd = ax[(ca + 1) % 2][:].rearrange("p (g w) -> p g w", g=2)
                nc.vector.tensor_copy(axd[:, 0, half * hw:(half + 1) * hw],
                                      ck[:, 0:hw])
                nc.vector.tensor_copy(axd[:, 1, half * hw:(half + 1) * hw],
                                      ck[:, hw:2 * hw])
                dst = xc[(ca + 1) % 2][:].rearrange("p (s w) -> p s w",
                                                    w=2 * BL)
                srch = ck[:, 2 * hw:3 * hw].rearrange("p (s w) -> p s w", w=BL)
                nc.vector.tensor_copy(dst[:, 4 * half:4 * half + 4, BL:2 * BL],
                                      srch[:, :, :])

            iT = pp.tile([128, B], BF16, tag="iT")

            def prep_item():
                # renorm all 256 target items, transpose into iT [D, B]
                for k in range(2):
                    itk = itemr[:, k * D:(k + 1) * D]
                    sqt = wk.tile([128, D], F32, tag="sqt")
                    sst = wk.tile([128, 1], F32, tag="sst")
                    nc.vector.scalar_tensor_tensor(
                        out=sqt[:], in0=itk, scalar=0.0, in1=itk,
                        op0=OP.add, op1=OP.mult, accum_out=sst[:])
                    sct = wk.tile([128, 1], F32, tag="sct")
                    _rsqrt(nc, wk, sst[:], sct[:], [128, 1], iters=2)
                    nc.vector.tensor_scalar_min(out=sct[:], in0=sct[:],
                                                scalar1=1.0)
                    itsc = wk.tile([128, D], BF16, tag="itsc")
                    nc.vector.tensor_scalar(out=itsc[:], in0=itk,
                                            scalar1=sct[:], scalar2=None,
                                            op0=OP.mult)
                    tp = pck.tile([128, 512], F32, tag="ck")
                    tpb = tp[:].bitcast(BF16)
                    nc.tensor.transpose(out=tpb[:, 0:D], in_=itsc[:],
                                        identity=identb[:])
                    nc.vector.tensor_copy(iT[:, k * D:(k + 1) * D],
                                          tpb[:, 0:D])

            def emit_gram(blk):
                gps = pck.tile([128, 512], F32, tag="ck")
                nc.tensor.matmul(gps[:, 0:128], ET[:, 128 * blk:128 * (blk + 1)],
                                 X_v[:, 4 * blk + 1:4 * blk + 5, 0:BL],
                                 start=True, stop=True)
                gsc = wk.tile([128, 128], F32, tag="gram_scr")
                nc.vector.scalar_tensor_tensor(
                    out=gsc[:], in0=gps[:, 0:128], scalar=1.0, in1=identf[:],
                    op0=OP.mult, op1=OP.mult,
                    accum_out=s_all[:, blk:blk + 1])

            # ---------- upfront pipeline fill ----------
            for k in range(8):
                do_tile(k)
            for j in range(6):
                do_tile2(j)
            emit_gx(0)
            emit_xn(0)

            if upto == "A":
                for k in range(8, NTIL):
                    do_tile(k)
                for j in range(6, NTIL):
                    do_tile2(j)
                dbg = wk.tile([1, 1], F32, tag="res")
                nc.vector.reduce_sum(out=dbg[:], in_=ET[0:1, 0:128],
                                     axis=mybir.AxisListType.X)
                nc.sync.dma_start(out=out_p[:], in_=dbg[:])

            # ---------- recurrence slot loop ----------
            # pg layouts: pgr = [rG|rA], pgg = [gG|uA]; gp = [rG rA gG uA]
            prev_p = prev_q = None
            if upto != "A":
                for s in range(NSLOT):
                    tg, ta, o, c = s, s - LAG, s % CH, s // CH
                    gru = tg < L
                    aug = 0 <= ta < L
                    if gru and aug:
                        cs = slice(0, 2 * BL)       # pair columns
                    elif gru:
                        cs = slice(0, BL)
                    else:
                        cs = slice(BL, 2 * BL)
                    gxc, axc, xcc = gx[c % 2], ax[c % 2], xc[c % 2]

                    # ---- per-step PSUM bank fill ----
                    pgr = ppgr.tile([128, 2 * BL], F32, tag="pgr")
                    pgg = ppgg.tile([128, 2 * BL], F32, tag="pgg")
                    pn = ppn.tile([128, 2 * BL], F32, tag="pn")
                    if gru:
                        gxv = gxc[:].rearrange("p (g w) -> p g w", g=2)
                        # injects: r-gate then update-gate x-preacts
                        nc.tensor.matmul(pgr[:, 0:BL], identb[:],
                                         gxv[:, 0, o * BL:(o + 1) * BL],
                                         start=True, stop=(s == 0))
                        nc.tensor.matmul(pgg[:, 0:BL], identb[:],
                                         gxv[:, 1, o * BL:(o + 1) * BL],
                                         start=True, stop=(s == 0))
                        nc.tensor.matmul(pn[:, 0:BL], bhhn_s[0:1, :],
                                         ones_row[0:1, 0:BL],
                                         start=True, stop=(s == 0))
                    if aug:
                        axv = axc[:].rearrange("p (g w) -> p g w", g=2)
                        nc.tensor.matmul(pgr[:, BL:2 * BL], identb[:],
                                         axv[:, 0, o * BL:(o + 1) * BL],
                                         start=True, stop=False)
                        nc.tensor.matmul(pgg[:, BL:2 * BL], identb[:],
                                         axv[:, 1, o * BL:(o + 1) * BL],
                                         start=True, stop=False)
                    if s == LAG:
                        # AUGRU step 0: h-side terms come straight from h0
                        nc.tensor.matmul(pgr[:, BL:2 * BL], auh_s[:, 0:D],
                                         h0s[:], start=False, stop=True)
                        nc.tensor.matmul(pgg[:, BL:2 * BL], auh_s[:, D:2 * D],
                                         h0s[:], start=False, stop=True)
                        nc.tensor.matmul(pn[:, BL:2 * BL], ahh_s[:], h0s[:],
                                         start=True, stop=True)
                    # q-side h-matmuls (prev_q ready during prev tanh window)
                    if s > 0 and gru:
                        qG = prev_q[:, 0:BL]
                        nc.tensor.matmul(pgr[:, 0:BL], wgh_s[:, 0:D], qG,
                                         start=False, stop=False)
                        nc.tensor.matmul(pgg[:, 0:BL], wgh_s[:, D:2 * D], qG,
                                         start=False, stop=False)
                        nc.tensor.matmul(pn[:, 0:BL], wnh_s[:], qG,
                                         start=False, stop=False)
                    if aug and ta > 0:
                        qA = prev_q[:, BL:2 * BL]
                        nc.tensor.matmul(pgr[:, BL:2 * BL], auh_s[:, 0:D], qA,
                                         start=False, stop=False)
                        nc.tensor.matmul(pgg[:, BL:2 * BL], auh_s[:, D:2 * D],
                                         qA, start=False, stop=False)
                        nc.tensor.matmul(pn[:, BL:2 * BL], ahh_s[:], qA,
                                         start=True, stop=False)
                    # HAM filler: PE activity while tanh/p run (no dst, no dep)
                    for _ in range(3):
                        nc.tensor.ldweights(weights=identb[:])

                    # ---- off-chain work: de-prioritized, fills idle ----
                    with tc.high_priority(offset=-DEPRI):
                        if o == 1 and c + 1 < NCHUNK:
                            emit_gx(c + 1)
                        if o == 3 and c + 1 < NCHUNK:
                            emit_xn(c + 1)
                        if o == 4 and c < NCHUNK:
                            emit_ah(c, 0)
                        if o == 0 and 1 <= c <= NCHUNK:
                            emit_ah(c - 1, 1)
                        if s >= 2 and (s - 2) % 4 == 3 and (s - 2) // 4 < NTIL \
                                and s - 2 < L:
                            emit_gram((s - 2) // 4)
                        if s % 2 == 1 and 8 + (s - 1) // 2 < NTIL:
                            do_tile(8 + (s - 1) // 2)
                        if s % 2 == 1 and s >= 5 and 6 + (s - 5) // 2 < NTIL:
                            do_tile2(6 + (s - 5) // 2)
                        if s == 120:
                            prep_item()

                    # p-side h-matmuls: r-gates first (they gate sigma_r)
                    if s > 0 and gru:
                        pG = prev_p[:, 0:BL]
                        nc.tensor.matmul(pgr[:, 0:BL], wgh_s[:, 0:D], pG,
                                         start=False, stop=True)
                    if aug and ta > 0:
                        pA = prev_p[:, BL:2 * BL]
                        nc.tensor.matmul(pgr[:, BL:2 * BL], auh_s[:, 0:D], pA,
                                         start=False, stop=True)
                    if s > 0 and gru:
                        nc.tensor.matmul(pn[:, 0:BL], wnh_s[:], pG,
                                         start=False, stop=True)
                    if aug and ta > 0:
                        nc.tensor.matmul(pn[:, BL:2 * BL], ahh_s[:], pA,
                                         start=False, stop=True)
                    if s > 0 and gru:
                        nc.tensor.matmul(pgg[:, 0:BL], wgh_s[:, D:2 * D], pG,
                                         start=False, stop=True)
                    if aug and ta > 0:
                        nc.tensor.matmul(pgg[:, BL:2 * BL], auh_s[:, D:2 * D],
                                         pA, start=False, stop=True)
                    # HAM filler: PE activity while sigma_r/tprod/t2 run
                    for _ in range(6):
                        nc.tensor.ldweights(weights=identb[:])

                    # ---- serial chain ----
                    gp = wk.tile([128, 4 * BL], BF16, tag="gp")
                    nc.scalar.activation(gp[:, cs], pgr[:, cs], AF.Sigmoid)
                    tprod = wk.tile([128, 2 * BL], BF16, tag="tprod")
                    nc.vector.tensor_tensor(out=tprod[:, cs], in0=pn[:, cs],
                                            in1=gp[:, cs], op=OP.mult)
                    t2 = wk.tile([128, 2 * BL], BF16, tag="t2")
                    t2i = nc.vector.tensor_tensor(
                        out=t2[:, cs], in0=tprod[:, cs],
                        in1=xcc[:, o * 2 * BL + cs.start:
                                o * 2 * BL + cs.stop],
                        op=OP.add)
                    gcs = slice(2 * BL + cs.start, 2 * BL + cs.stop)
                    nc.scalar.activation(gp[:, gcs], pgg[:, cs], AF.Sigmoid)
                    gT = wk.tile([128, 2 * BL], BF16, tag="gT")
                    nc.scalar.activation(gT[:, cs], t2[:, cs], AF.Tanh)
                    # during-tanh: zp = 1-g ; q = zp * h_prev
                    zp = wk.tile([128, 2 * BL], BF16, tag="zp")
                    zpi = nc.vector.tensor_scalar(out=zp[:, cs], in0=gp[:, gcs],
                                                  scalar1=-1.0, scalar2=1.0,
                                                  op0=OP.mult, op1=OP.add)
                    add_dep_helper(zpi.ins, t2i.ins, sync=False,
                                   reason="keep zp off the t2 chain")
                    q = pqp.tile([128, 2 * BL], BF16, tag="q")
                    nc.vector.tensor_tensor(out=q[:, cs], in0=zp[:, cs],
                                            in1=X_v[:, s, cs], op=OP.mult)
                    # post-tanh: p = g*T ; h' = p + q
                    p = pqp.tile([128, 2 * BL], BF16, tag="p")
                    nc.vector.tensor_tensor(out=p[:, cs], in0=gp[:, gcs],
                                            in1=gT[:, cs], op=OP.mult)
                    nc.vector.tensor_tensor(out=X_v[:, s + 1, cs],
                                            in0=p[:, cs], in1=q[:, cs],
                                            op=OP.add)
                    prev_p, prev_q = p, q

            if upto == "G":
                dbg = wk.tile([1, 1], F32, tag="res")
                dbf = wk.tile([1, BL], F32, tag="resb")
                nc.vector.tensor_copy(dbf[:], X_v[0:1, L, 0:BL])
                nc.vector.reduce_sum(out=dbg[:], in_=dbf[:],
                                     axis=mybir.AxisListType.X)
                nc.sync.dma_start(out=out_p[:], in_=dbg[:])
            if upto == "GA":
                dbg = wk.tile([1, 1], F32, tag="res")
                dbf = wk.tile([1, BL], F32, tag="resb")
                nc.vector.tensor_copy(dbf[:], X_v[0:1, NSLOT, BL:2 * BL])
                nc.vector.reduce_sum(out=dbg[:], in_=dbf[:],
                                     axis=mybir.AxisListType.X)
                nc.sync.dma_start(out=out_p[:], in_=dbg[:])

            if upto in ("X", "full"):
                # ---------- aux BCE partial: sp(s) + y*(sp(-s)-sp(s)) ------
                ebuf = pp.tile([128, NTIL], F32, tag="ebuf")
                nc.scalar.activation(ebuf[:], s_all[:], AF.Exp)
                nc.vector.tensor_scalar_add(out=ebuf[:], in0=ebuf[:],
                                            scalar1=1.0)
                sp = pp.tile([128, NTIL], F32, tag="sp")
                nc.scalar.activation(sp[:], ebuf[:], AF.Ln)
                spm = pp.tile([128, NTIL], F32, tag="spm")
                nc.vector.tensor_tensor(out=spm[:], in0=sp[:], in1=s_all[:],
                                        op=OP.subtract)
                nc.vector.tensor_scalar_min(out=spm[:], in0=spm[:],
                                            scalar1=100.0)
                nc.vector.tensor_scalar_min(out=sp[:], in0=sp[:], scalar1=100.0)
                nc.vector.tensor_tensor(out=spm[:], in0=spm[:], in1=sp[:],
                                        op=OP.subtract)
                nc.vector.tensor_tensor(out=spm[:], in0=y_h_s[:], in1=spm[:],
                                        op=OP.mult)
                nc.vector.tensor_tensor(out=sp[:], in0=sp[:], in1=spm[:],
                                        op=OP.add)
                rsum = wk.tile([128, 1], F32, tag="rsum")
                nc.vector.reduce_sum(out=rsum[:], in_=sp[:],
                                     axis=mybir.AxisListType.X)
                aux_ps = pck.tile([128, 512], F32, tag="ck")
                nc.tensor.matmul(aux_ps[0:1, 0:1], rsum[:], ones_col[:, 0:1],
                                 start=True, stop=True)

                # ---------- pack + AllGather (bf16) ----------
                stage = pp.tile([D, PW], BF16, tag="stage")
                nc.gpsimd.memset(stage[:], 0.0)
                nc.vector.tensor_copy(stage[:, 0:BL], X_v[:, NSLOT, BL:2 * BL])
                nc.vector.tensor_copy(stage[0:1, BL:BL + 1],
                                      aux_ps[0:1, 0:1])
                nc.sync.dma_start(out=ploc[:], in_=stage[:])
                nc.gpsimd.collective_compute(
                    "AllGather", OP.bypass,
                    replica_groups=[list(range(NCORES))],
                    ins=[ploc[:]], outs=[gall[:]],
                )

            if upto == "X":
                dbg = wk.tile([1, 1], F32, tag="res")
                nc.vector.tensor_copy(dbg[:], aux_ps[0:1, 0:1])
                nc.sync.dma_start(out=out_p[:], in_=dbg[:])

            if upto == "full":
                # ---------- replicated final MLP ----------
                gat = pp.tile([D, NCORES * PW], BF16, tag="gat")
                for cc in range(NCORES):
                    nc.sync.dma_start(out=gat[:, cc * PW:(cc + 1) * PW],
                                      in_=gall[cc * D:(cc + 1) * D, :])
                gat_v = gat[:].rearrange("p (c w) -> p c w", c=NCORES)
                hT_v = gat_v[:, :, 0:BL]            # [128, 8, 32]
                aux8 = wk.tile([1, NCORES], F32, tag="aux8")
                aux8_v = aux8[:].rearrange("p (c w) -> p c w", w=1)
                nc.vector.tensor_copy(aux8_v, gat_v[0:1, :, BL:BL + 1])
                aux_tot = wk.tile([1, 1], F32, tag="aux_tot")
                nc.vector.reduce_sum(out=aux_tot[:], in_=aux8[:],
                                     axis=mybir.AxisListType.X)

                def dice(z_ps, pdim):
                    m = wk.tile([pdim, 1], F32, tag="dice_m")
                    nc.vector.reduce_sum(out=m[:], in_=z_ps[:],
                                         axis=mybir.AxisListType.X)
                    nc.vector.tensor_scalar_mul(out=m[:], in0=m[:],
                                                scalar1=1.0 / B)
                    xcen = wk.tile([pdim, B], F32, tag="dice_xc")
                    nc.vector.tensor_scalar(out=xcen[:], in0=z_ps[:],
                                            scalar1=m[:], scalar2=None,
                                            op0=OP.subtract)
                    sq2 = wk.tile([pdim, B], F32, tag="dice_sq")
                    vs = wk.tile([pdim, 1], F32, tag="dice_vs")
                    nc.scalar.activation(sq2[:], xcen[:], AF.Square,
                                         accum_out=vs[:])
                    nc.vector.tensor_scalar(out=vs[:], in0=vs[:],
                                            scalar1=1.0 / B, scalar2=EPS_BN,
                                            op0=OP.mult, op1=OP.add)
                    inv = wk.tile([pdim, 1], F32, tag="dice_inv")
                    _rsqrt(nc, wk, vs[:], inv[:], [pdim, 1], iters=2)
                    pr = wk.tile([pdim, B], F32, tag="dice_p")
                    nc.scalar.activation(pr[:], xcen[:], AF.Sigmoid,
                                         scale=inv[:, 0:1])
                    nc.vector.tensor_scalar(out=pr[:], in0=pr[:],
                                            scalar1=1 - DICE_A, scalar2=DICE_A,
                                            op0=OP.mult, op1=OP.add)
                    zd = wk.tile([pdim, B], BF16, tag="dice_zd")
                    nc.vector.tensor_tensor(out=zd[:], in0=z_ps[:], in1=pr[:],
                                            op=OP.mult)
                    return zd

                z1_ps = pck.tile([128, 512], F32, tag="ck")
                nc.tensor.matmul(z1_ps[:, 0:B], W1a_s[:], hT_v,
                                 start=True, stop=False)
                nc.tensor.matmul(z1_ps[:, 0:B], W1b_s[:], iT[:, :],
                                 start=False, stop=False)
                nc.tensor.matmul(z1_ps[:, 0:B], b1_s[0:1, :], ones_b[0:1, :],
                                 start=False, stop=True)
                z1d = dice(z1_ps[:, 0:B], 128)

                z2_ps = pck.tile([128, 512], F32, tag="ck")
                nc.tensor.matmul(z2_ps[0:D // 2, 0:B], W2_s[:, :], z1d[:],
                                 start=True, stop=False)
                nc.tensor.matmul(z2_ps[0:D // 2, 0:B], b2_s[0:1, :],
                                 ones_b[0:1, :], start=False, stop=True)
                z2d = dice(z2_ps[0:D // 2, 0:B], D // 2)

                s_ps = pck.tile([128, 512], F32, tag="ck")
                nc.tensor.matmul(s_ps[0:1, 0:B], Wf_s[:, 0:1], z2d[:],
                                 start=True, stop=False)
                nc.tensor.matmul(s_ps[0:1, 0:B], bf_s[0:1, 0:1],
                                 ones_b[0:1, :], start=False, stop=True)
                s_sb = wk.tile([1, B], F32, tag="s_sb")
                nc.vector.tensor_copy(s_sb[:], s_ps[0:1, 0:B])

                e2 = wk.tile([1, B], F32, tag="e2")
                nc.scalar.activation(e2[:], s_sb[:], AF.Exp)
                nc.vector.tensor_scalar_add(out=e2[:], in0=e2[:], scalar1=1.0)
                sp2 = wk.tile([1, B], F32, tag="sp2")
                nc.scalar.activation(sp2[:], e2[:], AF.Ln)
                spm2 = wk.tile([1, B], F32, tag="spm2")
                nc.vector.tensor_tensor(out=spm2[:], in0=sp2[:], in1=s_sb[:],
                                        op=OP.subtract)
                nc.vector.tensor_scalar_min(out=spm2[:], in0=spm2[:],
                                            scalar1=100.0)
                nc.vector.tensor_scalar_min(out=sp2[:], in0=sp2[:],
                                            scalar1=100.0)
                nc.vector.tensor_tensor(out=spm2[:], in0=spm2[:], in1=sp2[:],
                                        op=OP.subtract)
                nc.vector.tensor_tensor(out=spm2[:], in0=y_t_s[:], in1=spm2[:],
                                        op=OP.mult)
                nc.vector.tensor_tensor(out=sp2[:], in0=sp2[:], in1=spm2[:],
                                        op=OP.add)
                rec_sum = wk.tile([1, 1], F32, tag="rec_sum")
                nc.vector.reduce_sum(out=rec_sum[:], in_=sp2[:],
                                     axis=mybir.AxisListType.X)

                nc.vector.tensor_scalar_mul(out=aux_tot[:], in0=aux_tot[:],
                                            scalar1=ALPHA / (B * L))
                nc.vector.tensor_scalar_mul(out=rec_sum[:], in0=rec_sum[:],
                                            scalar1=1.0 / B)
                res = wk.tile([1, 1], F32, tag="res")
                nc.vector.tensor_tensor(out=res[:], in0=aux_tot[:],
                                        in1=rec_sum[:], op=OP.add)
                nc.sync.dma_start(out=out_p[:], in_=res[:])
    nc.compile()
    return nc


_NC_CACHE = None


def _get_nc():
    global _NC_CACHE
    if _NC_CACHE is None:
        import os
        _NC_CACHE = build_bass(os.environ.get("KERNEL_UPTO", "full"))
    return _NC_CACHE


def _prep_inputs(inputs):
    f32 = np.float32
    import ml_dtypes
    bf16 = ml_dtypes.bfloat16
    emb = np.ascontiguousarray(inputs["emb"], dtype=f32)
    seqs = np.asarray(inputs["history_seqs"])
    labs = np.asarray(inputs["history_labels"])
    tgt = np.asarray(inputs["target_item"])
    tl = np.asarray(inputs["target_label"]).astype(f32)

    w_ih = np.asarray(inputs["w_ih"], dtype=f32)   # rows: [r | z | n]
    w_hh = np.asarray(inputs["w_hh"], dtype=f32)
    b_ih = np.asarray(inputs["b_ih"], dtype=f32)
    b_hh = np.asarray(inputs["b_hh"], dtype=f32)
    # gate order in banks: [r | zbar]; zbar = negated z
    wgx_m = np.concatenate([w_ih[0:D].T, -w_ih[D:2 * D].T], axis=1)
    wgh_m = np.concatenate([w_hh[0:D].T, -w_hh[D:2 * D].T], axis=1)
    bg = np.concatenate([b_ih[0:D] + b_hh[0:D],
                         -(b_ih[D:2 * D] + b_hh[D:2 * D])]).reshape(1, 2 * D)
    wnx_m = np.ascontiguousarray(w_ih[2 * D:3 * D].T)
    wnh_m = np.ascontiguousarray(w_hh[2 * D:3 * D].T)
    bihn = b_ih[2 * D:].reshape(1, D)
    bhhn = b_hh[2 * D:].reshape(1, D)

    Wu, Wr, Wh = (np.asarray(inputs[k], dtype=f32) for k in ("Wu", "Wr", "Wh"))
    Uu, Ur, Uh = (np.asarray(inputs[k], dtype=f32) for k in ("Uu", "Ur", "Uh"))
    bu = np.asarray(inputs["bu"], dtype=f32).reshape(-1)
    br = np.asarray(inputs["br"], dtype=f32).reshape(-1)
    bh = np.asarray(inputs["bh"], dtype=f32).reshape(1, D)
    aux_wm = np.concatenate([Wr, Wu], axis=1)       # [D, 2D], r first
    auh_wm = np.concatenate([Ur, Uu], axis=1)
    ba = np.concatenate([br, bu]).reshape(1, 2 * D)

    W1 = np.ascontiguousarray(inputs["W1"], dtype=f32)
    b1 = np.asarray(inputs["b1"], dtype=f32).reshape(1, D)
    W2 = np.ascontiguousarray(inputs["W2"], dtype=f32)
    b2 = np.asarray(inputs["b2"], dtype=f32).reshape(1, D // 2)
    Wf = np.ascontiguousarray(inputs["Wf"], dtype=f32)
    bfv = np.asarray(inputs["bf"], dtype=f32).reshape(1, 1)
    h0 = np.asarray(inputs["h0"], dtype=f32)
    y_t_full = tl.reshape(1, B)

    cvt = lambda a: np.ascontiguousarray(a).astype(bf16)
    shared = dict(
        emb=emb, wgx=cvt(wgx_m), wgh=cvt(wgh_m), wnx=cvt(wnx_m),
        wnh=cvt(wnh_m), bg_row=cvt(bg), bihn_r=cvt(bihn), bhhn_r=cvt(bhhn),
        aux_w=cvt(aux_wm), auh_w=cvt(auh_wm), ahx_w=cvt(Wh), ahh_w=cvt(Uh),
        ba_row=cvt(ba), bh_r=cvt(bh),
        W1a=cvt(W1[0:D]), W1b=cvt(W1[D:2 * D]),
        b1=cvt(b1), W2=cvt(W2), b2=cvt(b2), Wf=cvt(Wf), bf=cvt(bfv),
        y_t=y_t_full, idx_t=tgt.reshape(B, 1).astype(np.int32))
    in_maps = []
    for c in range(NCORES):
        sl = slice(c * BL, (c + 1) * BL)
        idx_f = np.ascontiguousarray(seqs[sl].T).reshape(-1)
        idx_hc = np.ascontiguousarray(
            idx_f.reshape(NTIL, 128).T).astype(np.int32)
        y_f = np.ascontiguousarray(labs[sl, :, 0].T).reshape(-1).astype(f32)
        y_hc = np.ascontiguousarray(y_f.reshape(NTIL, 128).T)
        h0Tc = cvt(h0[sl].T)
        m = dict(shared)
        m.update(idx_h=idx_hc, y_h=y_hc, h0T=h0Tc)
        in_maps.append(m)
    return in_maps


def kernel(**inputs) -> np.ndarray:
    nc = _get_nc()
    in_maps = _prep_inputs(inputs)
    res = run_bass_kernel_spmd(nc, in_maps, core_ids=list(range(NCORES)))
    out = np.asarray(res.results[0]["out"], dtype=np.float32)
    return out.reshape(())


# revision 46
# speedup vs baseline: 1.7148x; 1.1263x over previous
"""DIEN forward-loss kernel for Trainium2, SPMD over 8 NeuronCores. V3.

Data-parallel over batch (32 rows/core), embedding replicated. Critical-path
restructure vs V2: the per-step blend h' = g*T + (1-g)*h is split into
p = g*tanh(..) and q = (1-g)*h; since W@h' = W@p + W@q, the q-side h-matmuls
for the next step issue during the current tanh window and only the p-side
matmuls remain on the serial chain. Sigmoid is split into an early r-gate
sigmoid (gates the candidate product) and a later update-gate sigmoid (only
needed post-tanh), each in its own PSUM bank so they don't falsely couple.
The x-side preactivations land in per-step PSUM regions via identity-inject
matmuls from chunk-staged SBUF buffers; candidate x-terms are staged
interleaved ([xn_G | xh_A] per step) so t2 assembly is two DVE ops. GRU and
AUGRU run LAG=8 apart (one chunk) with AUGRU x-chunks emitted in half-chunks
to meet the tighter deadline. Aux-gram and AUGRU x-matmuls read the hidden
ring directly (strided rhs), no contiguous copy. BCE uses Softplus.

PSUM banks: pgr(x2) r-gate preacts [rG|rA], pgg(x2) update-gate preacts
[gG|uA], pn(x2) candidate h-side [nhG+bhhn|nhA], ck(x2) chunk-stage/
transposes/gram/MLP.
"""
import numpy as np
import concourse.bass as bass
import concourse.bacc as bacc
import concourse.mybir as mybir
import concourse.tile as tile
from concourse.tile import add_dep_helper
from concourse.bass_utils import run_bass_kernel_spmd
from concourse.masks import make_identity

F32 = mybir.dt.float32
BF16 = mybir.dt.bfloat16
I32 = mybir.dt.int32
AF = mybir.ActivationFunctionType
OP = mybir.AluOpType

B, L, D, NV = 256, 200, 128, 500000
NCORES = 8
BL = B // NCORES          # 32 batch rows per core
NT = L * BL               # 6400 (t,b) pairs per core
NTIL = NT // 128          # 50 gather tiles
CH = 8                    # timesteps per chunk
CW = CH * BL              # 256 cols per chunk gate region
NCHUNK = L // CH          # 25
LAG = CH                  # AUGRU lags GRU by one chunk
NSLOT = L + LAG           # 208
EPS_BN = 1e-5
DICE_A = 0.1
ALPHA = 0.2
MAGIC = 0x5F3759DF
PW = BL + 4               # 36: [hA(32) | aux | pad]
DEPRI = 10 ** 6           # scheduler de-priority offset for off-chain work


def _rsqrt(nc, pool, v, out, shape, iters=3):
    """out = 1/sqrt(v) on DVE (quake seed + Newton). v >= 0."""
    p, n = shape
    iv = out.bitcast(I32)
    nc.vector.tensor_scalar(out=iv, in0=v.bitcast(I32), scalar1=1,
                            scalar2=None, op0=OP.arith_shift_right)
    nc.vector.tensor_scalar(out=iv, in0=iv, scalar1=-1, scalar2=None,
                            op0=OP.bitwise_xor)
    nc.vector.tensor_scalar(out=iv, in0=iv, scalar1=MAGIC + 1, scalar2=None,
                            op0=OP.add)
    t = pool.tile([p, n], F32, tag="rsqrt_t")
    for _ in range(iters):
        nc.vector.tensor_tensor(out=t[:], in0=v, in1=out, op=OP.mult)
        nc.vector.tensor_tensor(out=t[:], in0=t[:], in1=out, op=OP.mult)
        nc.vector.tensor_scalar(out=t[:], in0=t[:], scalar1=-0.5, scalar2=1.5,
                                op0=OP.mult, op1=OP.add)
        nc.vector.tensor_tensor(out=out, in0=out, in1=t[:], op=OP.mult)


def build_bass(upto="full"):
    nc = bacc.Bacc("TRN2", target_bir_lowering=False, num_devices=NCORES)

    # ---------------- kernel parameters ----------------
    emb = nc.declare_dram_parameter("emb", [NV, D], F32, isOutput=False)
    idx_h = nc.declare_dram_parameter("idx_h", [128, NTIL], I32, isOutput=False)
    y_h = nc.declare_dram_parameter("y_h", [128, NTIL], F32, isOutput=False)
    idx_t = nc.declare_dram_parameter("idx_t", [B, 1], I32, isOutput=False)
    # GRU weights (bf16): x-side / h-side, gate order [r | zbar]
    wgx = nc.declare_dram_parameter("wgx", [D, 2 * D], BF16, isOutput=False)
    wgh = nc.declare_dram_parameter("wgh", [D, 2 * D], BF16, isOutput=False)
    wnx = nc.declare_dram_parameter("wnx", [D, D], BF16, isOutput=False)
    wnh = nc.declare_dram_parameter("wnh", [D, D], BF16, isOutput=False)
    bg_row = nc.declare_dram_parameter("bg_row", [1, 2 * D], BF16, isOutput=False)
    bihn_r = nc.declare_dram_parameter("bihn_r", [1, D], BF16, isOutput=False)
    bhhn_r = nc.declare_dram_parameter("bhhn_r", [1, D], BF16, isOutput=False)
    # AUGRU weights (bf16): gate order [r | u]
    aux_w = nc.declare_dram_parameter("aux_w", [D, 2 * D], BF16, isOutput=False)
    auh_w = nc.declare_dram_parameter("auh_w", [D, 2 * D], BF16, isOutput=False)
    ahx_w = nc.declare_dram_parameter("ahx_w", [D, D], BF16, isOutput=False)
    ahh_w = nc.declare_dram_parameter("ahh_w", [D, D], BF16, isOutput=False)
    ba_row = nc.declare_dram_parameter("ba_row", [1, 2 * D], BF16, isOutput=False)
    bh_r = nc.declare_dram_parameter("bh_r", [1, D], BF16, isOutput=False)
    # final MLP (bf16)
    W1a = nc.declare_dram_parameter("W1a", [D, D], BF16, isOutput=False)
    W1b = nc.declare_dram_parameter("W1b", [D, D], BF16, isOutput=False)
    b1 = nc.declare_dram_parameter("b1", [1, D], BF16, isOutput=False)
    W2 = nc.declare_dram_parameter("W2", [D, D // 2], BF16, isOutput=False)
    b2 = nc.declare_dram_parameter("b2", [1, D // 2], BF16, isOutput=False)
    Wf = nc.declare_dram_parameter("Wf", [D // 2, 1], BF16, isOutput=False)
    bf = nc.declare_dram_parameter("bf", [1, 1], BF16, isOutput=False)
    h0T = nc.declare_dram_parameter("h0T", [D, BL], BF16, isOutput=False)
    y_t = nc.declare_dram_parameter("y_t", [1, B], F32, isOutput=False)
    out_p = nc.declare_dram_parameter("out", [1, 1], F32, isOutput=True)

    ploc = nc.dram_tensor("ploc", [D, PW], BF16)
    gall = nc.dram_tensor("gall", [NCORES * D, PW], BF16)

    with tile.TileContext(nc) as tc:
        with (
            tc.tile_pool(name="persist", bufs=1) as pp,
            tc.tile_pool(name="work", bufs=3) as wk,
            tc.tile_pool(name="pq", bufs=3) as pqp,
            tc.tile_pool(name="ps_pgr", bufs=2, space="PSUM") as ppgr,
            tc.tile_pool(name="ps_pgg", bufs=2, space="PSUM") as ppgg,
            tc.tile_pool(name="ps_pn", bufs=2, space="PSUM") as ppn,
            tc.tile_pool(name="ps_ck", bufs=2, space="PSUM") as pck,
        ):
            # ---------------- index loads FIRST (gathers gate the head) ----
            idx_s = pp.tile([128, NTIL], I32, tag="idx_s")
            nc.sync.dma_start(out=idx_s[:], in_=idx_h[:])
            idx_t_s = pp.tile([128, 2], I32, tag="idx_t_s")
            idx_t_d = idx_t[:].rearrange("(k p) w -> p (k w)", k=2)
            nc.sync.dma_start(out=idx_t_s[:], in_=idx_t_d)

            # constants on the gpsimd queue BEFORE the gather stream
            identf = pp.tile([128, 128], F32, tag="identf")
            make_identity(nc, identf[:])
            identb = pp.tile([128, 128], BF16, tag="identb")
            nc.vector.tensor_copy(identb[:], identf[:])
            ones_row = pp.tile([1, CW], BF16, tag="ones_row")
            nc.gpsimd.memset(ones_row[:], 1.0)
            ones_b = pp.tile([1, B], BF16, tag="ones_b")
            nc.gpsimd.memset(ones_b[:], 1.0)
            ones_col = pp.tile([128, 1], F32, tag="ones_col")
            nc.gpsimd.memset(ones_col[:], 1.0)

            X = pp.tile([128, (NSLOT + 1) * 2 * BL], BF16, tag="X")
            X_v = X[:].rearrange("p (s w) -> p s w", w=2 * BL)
            nc.gpsimd.memset(X[:, 0:2 * BL], 0.0)
            stage = pp.tile([D, PW], BF16, tag="stage")
            nc.gpsimd.memset(stage[:], 0.0)

            erows = pp.tile([128, NT], F32, tag="erows")
            itemr = pp.tile([128, 2 * D], F32, tag="itemr")
            for k in range(NTIL):
                nc.gpsimd.indirect_dma_start(
                    out=erows[:, 128 * k:128 * (k + 1)], out_offset=None,
                    in_=emb[:],
                    in_offset=bass.IndirectOffsetOnAxis(ap=idx_s[:, k:k + 1],
                                                        axis=0),
                )
            # all 256 target items gathered locally (replicated)
            for k in range(2):
                nc.gpsimd.indirect_dma_start(
                    out=itemr[:, k * D:(k + 1) * D], out_offset=None,
                    in_=emb[:],
                    in_offset=bass.IndirectOffsetOnAxis(
                        ap=idx_t_s[:, k:k + 1], axis=0),
                )

            def load(ap, shape, tag, dt=BF16):
                t = pp.tile(shape, dt, tag=tag)
                nc.sync.dma_start(out=t[:], in_=ap[:])
                return t

            wgx_s = load(wgx, [D, 2 * D], "wgx")
            wgh_s = load(wgh, [D, 2 * D], "wgh")
            wnx_s = load(wnx, [D, D], "wnx")
            wnh_s = load(wnh, [D, D], "wnh")
            bg_s = load(bg_row, [1, 2 * D], "bg")
            bihn_s = load(bihn_r, [1, D], "bihn")
            bhhn_s = load(bhhn_r, [1, D], "bhhn")
            aux_s = load(aux_w, [D, 2 * D], "aux")
            auh_s = load(auh_w, [D, 2 * D], "auh")
            ahx_s = load(ahx_w, [D, D], "ahx")
            ahh_s = load(ahh_w, [D, D], "ahh")
            ba_s = load(ba_row, [1, 2 * D], "ba")
            bh_s = load(bh_r, [1, D], "bh")
            W1a_s = load(W1a, [D, D], "W1a")
            W1b_s = load(W1b, [D, D], "W1b")
            b1_s = load(b1, [1, D], "b1")
            W2_s = load(W2, [D, D // 2], "W2")
            b2_s = load(b2, [1, D // 2], "b2")
            Wf_s = load(Wf, [D // 2, 1], "Wf")
            bf_s = load(bf, [1, 1], "bf")
            y_t_s = load(y_t, [1, B], "y_t", F32)
            y_h_s = load(y_h, [128, NTIL], "y_h", F32)
            h0s = load(h0T, [D, BL], "h0s")

            # persistent big buffers
            ET = pp.tile([128, NT], BF16, tag="ET")
            ss_all = pp.tile([128, NTIL], F32, tag="ss_all")
            sc_all = pp.tile([128, NTIL], F32, tag="sc_all")
            s_all = pp.tile([128, NTIL], F32, tag="s_all")
            # chunk staging buffers (manual double-buffer)
            gx0 = pp.tile([128, 2 * CW], BF16, tag="gx0")
            gx1 = pp.tile([128, 2 * CW], BF16, tag="gx1")
            ax0 = pp.tile([128, 2 * CW], BF16, tag="ax0")
            ax1 = pp.tile([128, 2 * CW], BF16, tag="ax1")
            # candidate x-terms, interleaved [xn|xh] per step, split per
            # half-chunk so the late xh writes never touch a tile being read
            xcA0 = pp.tile([128, CW], BF16, tag="xcA0")
            xcA1 = pp.tile([128, CW], BF16, tag="xcA1")
            xcB0 = pp.tile([128, CW], BF16, tag="xcB0")
            xcB1 = pp.tile([128, CW], BF16, tag="xcB1")
            gx, ax = [gx0, gx1], [ax0, ax1]
            xcA, xcB = [xcA0, xcA1], [xcB0, xcB1]

            # hA(-1) = h0, read by slot LAG
            nc.vector.tensor_copy(X_v[:, LAG, BL:2 * BL], h0s[:])

            # ---------- phase A: norm + transpose pipeline ----------
            def do_tile(k):
                er = erows[:, 128 * k:128 * (k + 1)]
                sq = wk.tile([128, 128], F32, tag="sq_scr")
                nc.vector.scalar_tensor_tensor(
                    out=sq[:], in0=er, scalar=0.0, in1=er,
                    op0=OP.add, op1=OP.mult, accum_out=ss_all[:, k:k + 1])
                if k % 4 == 3 or k == NTIL - 1:
                    k0 = (k // 4) * 4
                    w = k - k0 + 1
                    _rsqrt(nc, wk, ss_all[:, k0:k + 1], sc_all[:, k0:k + 1],
                           [128, w], iters=1)
                    nc.vector.tensor_scalar_min(out=sc_all[:, k0:k + 1],
                                                in0=sc_all[:, k0:k + 1],
                                                scalar1=1.0)

            def do_tile2(j):
                erj = erows[:, 128 * j:128 * (j + 1)]
                ersc = wk.tile([128, 128], BF16, tag="ersc")
                nc.vector.tensor_scalar(out=ersc[:], in0=erj,
                                        scalar1=sc_all[:, j:j + 1],
                                        scalar2=None, op0=OP.mult)
                tp = pck.tile([128, 512], F32, tag="ck")
                tpb = tp[:].bitcast(BF16)
                nc.tensor.transpose(out=tpb[:, 0:128], in_=ersc[:],
                                    identity=identb[:])
                nc.vector.tensor_copy(ET[:, 128 * j:128 * (j + 1)],
                                      tpb[:, 0:128])

            # ---------- chunk emissions ----------
            def emit_gx(c):
                # GRU x-gate preacts for chunk c -> gx[c%2]: [rx(256)|zbx(256)]
                ecols = ET[:, c * CW:(c + 1) * CW]
                hw = CW // 2
                ck = pck.tile([128, 512], F32, tag="ck")
                for g in range(2):
                    w = wgx_s[:, g * D:(g + 1) * D]
                    bgg = bg_s[0:1, g * D:(g + 1) * D]
                    for h in range(2):
                        slc = slice(g * CW + h * hw, g * CW + (h + 1) * hw)
                        esl = ecols[:, h * hw:(h + 1) * hw]
                        nc.tensor.matmul(ck[:, slc], w, esl,
                                         start=True, stop=False)
                        nc.tensor.matmul(ck[:, slc], bgg, ones_row[0:1, 0:hw],
                                         start=False, stop=True)
                        nc.vector.tensor_copy(gx[c % 2][:, slc], ck[:, slc])

            def emit_xn(c):
                # GRU candidate x-preacts chunk c -> xcA/xcB interleaved slots
                ecols = ET[:, c * CW:(c + 1) * CW]
                hw = CW // 2
                ck = pck.tile([128, 512], F32, tag="ck")
                for h in range(2):
                    slc = slice(h * hw, (h + 1) * hw)
                    nc.tensor.matmul(ck[:, slc], wnx_s[:],
                                     ecols[:, slc], start=True, stop=False)
                    nc.tensor.matmul(ck[:, slc], bihn_s[0:1, :],
                                     ones_row[0:1, 0:hw], start=False,
                                     stop=True)
                    xdst = (xcA if h == 0 else xcB)[c % 2]
                    dst = xdst[:].rearrange("p (s w) -> p s w", w=2 * BL)
                    src = ck[:, slc].rearrange("p (s w) -> p s w", w=BL)
                    nc.vector.tensor_copy(dst[:, :, 0:BL], src[:, :, :])

            def emit_ah(ca, half):
                # AUGRU x-preacts for chunk ca, steps 4*half..4*half+3.
                # Gates -> ax[(ca+1)%2] ([rx|ux]), candidate -> xcA/xcB.
                t0 = ca * CH + 4 * half
                ocols = X_v[:, t0 + 1:t0 + 5, 0:BL]      # outs t0..t0+3
                hw = 4 * BL                              # 128
                ck = pck.tile([128, 512], F32, tag="ck")
                axd = ax[(ca + 1) % 2][:].rearrange("p (g w) -> p g w", g=2)
                for g in range(2):
                    slc = slice(g * hw, (g + 1) * hw)
                    nc.tensor.matmul(ck[:, slc], aux_s[:, g * D:(g + 1) * D],
                                     ocols, start=True, stop=False)
                    nc.tensor.matmul(ck[:, slc], ba_s[0:1, g * D:(g + 1) * D],
                                     ones_row[0:1, 0:hw], start=False,
                                     stop=True)
                    nc.vector.tensor_copy(
                        axd[:, g, half * hw:(half + 1) * hw], ck[:, slc])
                nc.tensor.matmul(ck[:, 2 * hw:3 * hw], ahx_s[:], ocols,
                                 start=True, stop=False)
                nc.tensor.matmul(ck[:, 2 * hw:3 * hw], bh_s[0:1, :],
                                 ones_row[0:1, 0:hw], start=False, stop=True)
                xdst = (xcA if half == 0 else xcB)[(ca + 1) % 2]
                dst = xdst[:].rearrange("p (s w) -> p s w", w=2 * BL)
                srch = ck[:, 2 * hw:3 * hw].rearrange("p (s w) -> p s w", w=BL)
                nc.vector.tensor_copy(dst[:, :, BL:2 * BL], srch[:, :, :])

            iT = pp.tile([128, B], BF16, tag="iT")

            def prep_item():
                # renorm all 256 target items, transpose into iT [D, B]
                for k in range(2):
                    itk = itemr[:, k * D:(k + 1) * D]
                    sqt = wk.tile([128, D], F32, tag="sqt")
                    sst = wk.tile([128, 1], F32, tag="sst")
                    nc.vector.scalar_tensor_tensor(
                        out=sqt[:], in0=itk, scalar=0.0, in1=itk,
                        op0=OP.add, op1=OP.mult, accum_out=sst[:])
                    sct = wk.tile([128, 1], F32, tag="sct")
                    _rsqrt(nc, wk, sst[:], sct[:], [128, 1], iters=2)
                    nc.vector.tensor_scalar_min(out=sct[:], in0=sct[:],
                                                scalar1=1.0)
                    itsc = wk.tile([128, D], BF16, tag="itsc")
                    nc.vector.tensor_scalar(out=itsc[:], in0=itk,
                                            scalar1=sct[:], scalar2=None,
                                            op0=OP.mult)
                    tp = pck.tile([128, 512], F32, tag="ck")
                    tpb = tp[:].bitcast(BF16)
                    nc.tensor.transpose(out=tpb[:, 0:D], in_=itsc[:],
                                        identity=identb[:])
                    nc.vector.tensor_copy(iT[:, k * D:(k + 1) * D],
                                          tpb[:, 0:D])

            def do_aux_bce():
                # aux BCE partial: sp(s) + y*(sp(-s)-sp(s)), summed
                ebuf = pp.tile([128, NTIL], F32, tag="ebuf")
                nc.scalar.activation(ebuf[:], s_all[:], AF.Exp)
                nc.vector.tensor_scalar_add(out=ebuf[:], in0=ebuf[:],
                                            scalar1=1.0)
                sp = pp.tile([128, NTIL], F32, tag="sp")
                nc.scalar.activation(sp[:], ebuf[:], AF.Ln)
                spm = pp.tile([128, NTIL], F32, tag="spm")
                nc.vector.tensor_tensor(out=spm[:], in0=sp[:], in1=s_all[:],
                                        op=OP.subtract)
                nc.vector.tensor_scalar_min(out=spm[:], in0=spm[:],
                                            scalar1=100.0)
                nc.vector.tensor_scalar_min(out=sp[:], in0=sp[:],
                                            scalar1=100.0)
                nc.vector.tensor_tensor(out=spm[:], in0=spm[:], in1=sp[:],
                                        op=OP.subtract)
                nc.vector.tensor_tensor(out=spm[:], in0=y_h_s[:], in1=spm[:],
                                        op=OP.mult)
                nc.vector.tensor_tensor(out=sp[:], in0=sp[:], in1=spm[:],
                                        op=OP.add)
                rsum = wk.tile([128, 1], F32, tag="rsum")
                nc.vector.reduce_sum(out=rsum[:], in_=sp[:],
                                     axis=mybir.AxisListType.X)
                aps = pck.tile([128, 512], F32, tag="ck")
                nc.tensor.matmul(aps[0:1, 0:1], rsum[:], ones_col[:, 0:1],
                                 start=True, stop=True)
                return aps

            def emit_gram(blk):
                gps = pck.tile([128, 512], F32, tag="ck")
                nc.tensor.matmul(gps[:, 0:128], ET[:, 128 * blk:128 * (blk + 1)],
                                 X_v[:, 4 * blk + 1:4 * blk + 5, 0:BL],
                                 start=True, stop=True)
                gsc = wk.tile([128, 128], F32, tag="gram_scr")
                nc.vector.scalar_tensor_tensor(
                    out=gsc[:], in0=gps[:, 0:128], scalar=1.0, in1=identf[:],
                    op0=OP.mult, op1=OP.mult,
                    accum_out=s_all[:, blk:blk + 1])

            # ---------- upfront pipeline fill ----------
            for k in range(8):
                do_tile(k)
            for j in range(6):
                do_tile2(j)
            emit_gx(0)
            emit_xn(0)

            if upto == "A":
                for k in range(8, NTIL):
                    do_tile(k)
                for j in range(6, NTIL):
                    do_tile2(j)
                dbg = wk.tile([1, 1], F32, tag="res")
                nc.vector.reduce_sum(out=dbg[:], in_=ET[0:1, 0:128],
                                     axis=mybir.AxisListType.X)
                nc.sync.dma_start(out=out_p[:], in_=dbg[:])

            # ---------- recurrence slot loop ----------
            # pg layouts: pgr = [rG|rA], pgg = [gG|uA]; gp = [rG rA gG uA]
            prev_p = prev_q = None
            if upto != "A":
                for s in range(NSLOT):
                    tg, ta, o, c = s, s - LAG, s % CH, s // CH
                    gru = tg < L
                    aug = 0 <= ta < L
                    if gru and aug:
                        cs = slice(0, 2 * BL)       # pair columns
                    elif gru:
                        cs = slice(0, BL)
                    else:
                        cs = slice(BL, 2 * BL)
                    gxc, axc = gx[c % 2], ax[c % 2]
                    xcc = (xcA if o < 4 else xcB)[c % 2]
                    oh = o % 4

                    # ---- per-step PSUM bank fill ----
                    pgr = ppgr.tile([128, 2 * BL], F32, tag="pgr")
                    pgg = ppgg.tile([128, 2 * BL], F32, tag="pgg")
                    pn = ppn.tile([128, 2 * BL], F32, tag="pn")
                    if gru:
                        gxv = gxc[:].rearrange("p (g w) -> p g w", g=2)
                        # injects: r-gate then update-gate x-preacts
                        nc.tensor.matmul(pgr[:, 0:BL], identb[:],
                                         gxv[:, 0, o * BL:(o + 1) * BL],
                                         start=True, stop=(s == 0))
                        nc.tensor.matmul(pgg[:, 0:BL], identb[:],
                                         gxv[:, 1, o * BL:(o + 1) * BL],
                                         start=True, stop=(s == 0))
                        nc.tensor.matmul(pn[:, 0:BL], bhhn_s[0:1, :],
                                         ones_row[0:1, 0:BL],
                                         start=True, stop=(s == 0))
                    if aug:
                        axv = axc[:].rearrange("p (g w) -> p g w", g=2)
                        nc.tensor.matmul(pgr[:, BL:2 * BL], identb[:],
                                         axv[:, 0, o * BL:(o + 1) * BL],
                                         start=True, stop=False)
                        nc.tensor.matmul(pgg[:, BL:2 * BL], identb[:],
                                         axv[:, 1, o * BL:(o + 1) * BL],
                                         start=True, stop=False)
                    if s == LAG:
                        # AUGRU step 0: h-side terms come straight from h0
                        nc.tensor.matmul(pgr[:, BL:2 * BL], auh_s[:, 0:D],
                                         h0s[:], start=False, stop=True)
                        nc.tensor.matmul(pgg[:, BL:2 * BL], auh_s[:, D:2 * D],
                                         h0s[:], start=False, stop=True)
                        nc.tensor.matmul(pn[:, BL:2 * BL], ahh_s[:], h0s[:],
                                         start=True, stop=True)
                    # q-side h-matmuls (prev_q ready during prev tanh window)
                    if s > 0 and gru:
                        qG = prev_q[:, 0:BL]
                        nc.tensor.matmul(pgr[:, 0:BL], wgh_s[:, 0:D], qG,
                                         start=False, stop=False)
                        nc.tensor.matmul(pgg[:, 0:BL], wgh_s[:, D:2 * D], qG,
                                         start=False, stop=False)
                        nc.tensor.matmul(pn[:, 0:BL], wnh_s[:], qG,
                                         start=False, stop=False)
                    if aug and ta > 0:
                        qA = prev_q[:, BL:2 * BL]
                        nc.tensor.matmul(pgr[:, BL:2 * BL], auh_s[:, 0:D], qA,
                                         start=False, stop=False)
                        nc.tensor.matmul(pgg[:, BL:2 * BL], auh_s[:, D:2 * D],
                                         qA, start=False, stop=False)
                        nc.tensor.matmul(pn[:, BL:2 * BL], ahh_s[:], qA,
                                         start=True, stop=False)

                    # ---- off-chain work: de-prioritized, fills idle ----
                    with tc.high_priority(offset=-DEPRI):
                        if o == 1 and c + 1 < NCHUNK:
                            emit_gx(c + 1)
                        if o == 3 and c + 1 < NCHUNK:
                            emit_xn(c + 1)
                        if o == 4 and c < NCHUNK:
                            emit_ah(c, 0)
                        if o == 0 and 1 <= c <= NCHUNK:
                            emit_ah(c - 1, 1)
                        if s >= 2 and (s - 2) % 4 == 3 and (s - 2) // 4 < NTIL \
                                and s - 2 < L:
                            emit_gram((s - 2) // 4)
                        if s % 2 == 1 and 8 + (s - 1) // 2 < NTIL:
                            do_tile(8 + (s - 1) // 2)
                        if s % 2 == 1 and s >= 5 and 6 + (s - 5) // 2 < NTIL:
                            do_tile2(6 + (s - 5) // 2)
                        if s == 120:
                            prep_item()
                        if s == 204 and upto in ("X", "full"):
                            aux_ps = do_aux_bce()

                    # p-side h-matmuls: r-gates first (they gate sigma_r)
                    if s > 0 and gru:
                        pG = prev_p[:, 0:BL]
                        nc.tensor.matmul(pgr[:, 0:BL], wgh_s[:, 0:D], pG,
                                         start=False, stop=True)
                    if aug and ta > 0:
                        pA = prev_p[:, BL:2 * BL]
                        nc.tensor.matmul(pgr[:, BL:2 * BL], auh_s[:, 0:D], pA,
                                         start=False, stop=True)
                    if s > 0 and gru:
                        nc.tensor.matmul(pn[:, 0:BL], wnh_s[:], pG,
                                         start=False, stop=True)
                    if aug and ta > 0:
                        nc.tensor.matmul(pn[:, BL:2 * BL], ahh_s[:], pA,
                                         start=False, stop=True)
                    if s > 0 and gru:
                        nc.tensor.matmul(pgg[:, 0:BL], wgh_s[:, D:2 * D], pG,
                                         start=False, stop=True)
                    if aug and ta > 0:
                        nc.tensor.matmul(pgg[:, BL:2 * BL], auh_s[:, D:2 * D],
                                         pA, start=False, stop=True)

                    # ---- serial chain ----
                    gp = wk.tile([128, 4 * BL], BF16, tag="gp")
                    nc.scalar.activation(gp[:, cs], pgr[:, cs], AF.Sigmoid)
                    tprod = wk.tile([128, 2 * BL], BF16, tag="tprod")
                    nc.vector.tensor_tensor(out=tprod[:, cs], in0=pn[:, cs],
                                            in1=gp[:, cs], op=OP.mult)
                    t2 = wk.tile([128, 2 * BL], BF16, tag="t2")
                    t2i = nc.vector.tensor_tensor(
                        out=t2[:, cs], in0=tprod[:, cs],
                        in1=xcc[:, oh * 2 * BL + cs.start:
                                oh * 2 * BL + cs.stop],
                        op=OP.add)
                    gcs = slice(2 * BL + cs.start, 2 * BL + cs.stop)
                    nc.scalar.activation(gp[:, gcs], pgg[:, cs], AF.Sigmoid)
                    gT = wk.tile([128, 2 * BL], BF16, tag="gT")
                    nc.scalar.activation(gT[:, cs], t2[:, cs], AF.Tanh)
                    # during-tanh: zp = 1-g ; q = zp * h_prev
                    zp = wk.tile([128, 2 * BL], BF16, tag="zp")
                    zpi = nc.vector.tensor_scalar(out=zp[:, cs], in0=gp[:, gcs],
                                                  scalar1=-1.0, scalar2=1.0,
                                                  op0=OP.mult, op1=OP.add)
                    add_dep_helper(zpi.ins, t2i.ins, sync=False,
                                   reason="keep zp off the t2 chain")
                    q = pqp.tile([128, 2 * BL], BF16, tag="q")
                    nc.vector.tensor_tensor(out=q[:, cs], in0=zp[:, cs],
                                            in1=X_v[:, s, cs], op=OP.mult)
                    # post-tanh: p = g*T ; h' = p + q
                    p = pqp.tile([128, 2 * BL], BF16, tag="p")
                    nc.vector.tensor_tensor(out=p[:, cs], in0=gp[:, gcs],
                                            in1=gT[:, cs], op=OP.mult)
                    nc.vector.tensor_tensor(out=X_v[:, s + 1, cs],
                                            in0=p[:, cs], in1=q[:, cs],
                                            op=OP.add)
                    prev_p, prev_q = p, q

            if upto == "G":
                dbg = wk.tile([1, 1], F32, tag="res")
                dbf = wk.tile([1, BL], F32, tag="resb")
                nc.vector.tensor_copy(dbf[:], X_v[0:1, L, 0:BL])
                nc.vector.reduce_sum(out=dbg[:], in_=dbf[:],
                                     axis=mybir.AxisListType.X)
                nc.sync.dma_start(out=out_p[:], in_=dbg[:])
            if upto == "GA":
                dbg = wk.tile([1, 1], F32, tag="res")
                dbf = wk.tile([1, BL], F32, tag="resb")
                nc.vector.tensor_copy(dbf[:], X_v[0:1, NSLOT, BL:2 * BL])
                nc.vector.reduce_sum(out=dbg[:], in_=dbf[:],
                                     axis=mybir.AxisListType.X)
                nc.sync.dma_start(out=out_p[:], in_=dbg[:])

            if upto in ("X", "full"):
                # ---------- pack + AllGather (bf16) ----------
                nc.vector.tensor_copy(stage[:, 0:BL], X_v[:, NSLOT, BL:2 * BL])
                nc.vector.tensor_copy(stage[0:1, BL:BL + 1],
                                      aux_ps[0:1, 0:1])
                nc.sync.dma_start(out=ploc[:], in_=stage[:])
                nc.gpsimd.collective_compute(
                    "AllGather", OP.bypass,
                    replica_groups=[list(range(NCORES))],
                    ins=[ploc[:]], outs=[gall[:]],
                )

            if upto == "X":
                dbg = wk.tile([1, 1], F32, tag="res")
                nc.vector.tensor_copy(dbg[:], aux_ps[0:1, 0:1])
                nc.sync.dma_start(out=out_p[:], in_=dbg[:])

            if upto == "full":
                # ---------- replicated final MLP ----------
                gat = pp.tile([D, NCORES * PW], BF16, tag="gat")
                for cc in range(NCORES):
                    nc.sync.dma_start(out=gat[:, cc * PW:(cc + 1) * PW],
                                      in_=gall[cc * D:(cc + 1) * D, :])
                gat_v = gat[:].rearrange("p (c w) -> p c w", c=NCORES)
                hT_v = gat_v[:, :, 0:BL]            # [128, 8, 32]
                aux8 = wk.tile([1, NCORES], F32, tag="aux8")
                aux8_v = aux8[:].rearrange("p (c w) -> p c w", w=1)
                nc.vector.tensor_copy(aux8_v, gat_v[0:1, :, BL:BL + 1])
                aux_tot = wk.tile([1, 1], F32, tag="aux_tot")
                nc.vector.reduce_sum(out=aux_tot[:], in_=aux8[:],
                                     axis=mybir.AxisListType.X)

                def dice(z_ps, pdim):
                    m = wk.tile([pdim, 1], F32, tag="dice_m")
                    nc.vector.reduce_sum(out=m[:], in_=z_ps[:],
                                         axis=mybir.AxisListType.X)
                    nc.vector.tensor_scalar_mul(out=m[:], in0=m[:],
                                                scalar1=1.0 / B)
                    xcen = wk.tile([pdim, B], F32, tag="dice_xc")
                    nc.vector.tensor_scalar(out=xcen[:], in0=z_ps[:],
                                            scalar1=m[:], scalar2=None,
                                            op0=OP.subtract)
                    sq2 = wk.tile([pdim, B], F32, tag="dice_sq")
                    vs = wk.tile([pdim, 1], F32, tag="dice_vs")
                    nc.scalar.activation(sq2[:], xcen[:], AF.Square,
                                         accum_out=vs[:])
                    nc.vector.tensor_scalar(out=vs[:], in0=vs[:],
                                            scalar1=1.0 / B, scalar2=EPS_BN,
                                            op0=OP.mult, op1=OP.add)
                    inv = wk.tile([pdim, 1], F32, tag="dice_inv")
                    _rsqrt(nc, wk, vs[:], inv[:], [pdim, 1], iters=2)
                    pr = wk.tile([pdim, B], F32, tag="dice_p")
                    nc.scalar.activation(pr[:], xcen[:], AF.Sigmoid,
                                         scale=inv[:, 0:1])
                    nc.vector.tensor_scalar(out=pr[:], in0=pr[:],
                                            scalar1=1 - DICE_A, scalar2=DICE_A,
                                            op0=OP.mult, op1=OP.add)
                    zd = wk.tile([pdim, B], BF16, tag="dice_zd")
                    nc.vector.tensor_tensor(out=zd[:], in0=z_ps[:], in1=pr[:],
                                            op=OP.mult)
                    return zd

                z1_ps = pck.tile([128, 512], F32, tag="ck")
                nc.tensor.matmul(z1_ps[:, 0:B], W1a_s[:], hT_v,
                                 start=True, stop=False)
                nc.tensor.matmul(z1_ps[:, 0:B], W1b_s[:], iT[:, :],
                                 start=False, stop=False)
                nc.tensor.matmul(z1_ps[:, 0:B], b1_s[0:1, :], ones_b[0:1, :],
                                 start=False, stop=True)
                z1d = dice(z1_ps[:, 0:B], 128)

                z2_ps = pck.tile([128, 512], F32, tag="ck")
                nc.tensor.matmul(z2_ps[0:D // 2, 0:B], W2_s[:, :], z1d[:],
                                 start=True, stop=False)
                nc.tensor.matmul(z2_ps[0:D // 2, 0:B], b2_s[0:1, :],
                                 ones_b[0:1, :], start=False, stop=True)
                z2d = dice(z2_ps[0:D // 2, 0:B], D // 2)

                s_ps = pck.tile([128, 512], F32, tag="ck")
                nc.tensor.matmul(s_ps[0:1, 0:B], Wf_s[:, 0:1], z2d[:],
                                 start=True, stop=False)
                nc.tensor.matmul(s_ps[0:1, 0:B], bf_s[0:1, 0:1],
                                 ones_b[0:1, :], start=False, stop=True)
                s_sb = wk.tile([1, B], F32, tag="s_sb")
                nc.vector.tensor_copy(s_sb[:], s_ps[0:1, 0:B])

                e2 = wk.tile([1, B], F32, tag="e2")
                nc.scalar.activation(e2[:], s_sb[:], AF.Exp)
                nc.vector.tensor_scalar_add(out=e2[:], in0=e2[:], scalar1=1.0)
                sp2 = wk.tile([1, B], F32, tag="sp2")
                nc.scalar.activation(sp2[:], e2[:], AF.Ln)
                spm2 = wk.tile([1, B], F32, tag="spm2")
                nc.vector.tensor_tensor(out=spm2[:], in0=sp2[:], in1=s_sb[:],
                                        op=OP.subtract)
                nc.vector.tensor_scalar_min(out=spm2[:], in0=spm2[:],
                                            scalar1=100.0)
                nc.vector.tensor_scalar_min(out=sp2[:], in0=sp2[:],
                                            scalar1=100.0)
                nc.vector.tensor_tensor(out=spm2[:], in0=spm2[:], in1=sp2[:],
                                        op=OP.subtract)
                nc.vector.tensor_tensor(out=spm2[:], in0=y_t_s[:], in1=spm2[:],
                                        op=OP.mult)
                nc.vector.tensor_tensor(out=sp2[:], in0=sp2[:], in1=spm2[:],
                                        op=OP.add)
                rec_sum = wk.tile([1, 1], F32, tag="rec_sum")
                nc.vector.reduce_sum(out=rec_sum[:], in_=sp2[:],
                                     axis=mybir.AxisListType.X)

                nc.vector.tensor_scalar_mul(out=aux_tot[:], in0=aux_tot[:],
                                            scalar1=ALPHA / (B * L))
                nc.vector.tensor_scalar_mul(out=rec_sum[:], in0=rec_sum[:],
                                            scalar1=1.0 / B)
                res = wk.tile([1, 1], F32, tag="res")
                nc.vector.tensor_tensor(out=res[:], in0=aux_tot[:],
                                        in1=rec_sum[:], op=OP.add)
                nc.sync.dma_start(out=out_p[:], in_=res[:])
    nc.compile()
    return nc


_NC_CACHE = None


def _get_nc():
    global _NC_CACHE
    if _NC_CACHE is None:
        import os
        _NC_CACHE = build_bass(os.environ.get("KERNEL_UPTO", "full"))
    return _NC_CACHE


def _prep_inputs(inputs):
    f32 = np.float32
    import ml_dtypes
    bf16 = ml_dtypes.bfloat16
    emb = np.ascontiguousarray(inputs["emb"], dtype=f32)
    seqs = np.asarray(inputs["history_seqs"])
    labs = np.asarray(inputs["history_labels"])
    tgt = np.asarray(inputs["target_item"])
    tl = np.asarray(inputs["target_label"]).astype(f32)

    w_ih = np.asarray(inputs["w_ih"], dtype=f32)   # rows: [r | z | n]
    w_hh = np.asarray(inputs["w_hh"], dtype=f32)
    b_ih = np.asarray(inputs["b_ih"], dtype=f32)
    b_hh = np.asarray(inputs["b_hh"], dtype=f32)
    # gate order in banks: [r | zbar]; zbar = negated z
    wgx_m = np.concatenate([w_ih[0:D].T, -w_ih[D:2 * D].T], axis=1)
    wgh_m = np.concatenate([w_hh[0:D].T, -w_hh[D:2 * D].T], axis=1)
    bg = np.concatenate([b_ih[0:D] + b_hh[0:D],
                         -(b_ih[D:2 * D] + b_hh[D:2 * D])]).reshape(1, 2 * D)
    wnx_m = np.ascontiguousarray(w_ih[2 * D:3 * D].T)
    wnh_m = np.ascontiguousarray(w_hh[2 * D:3 * D].T)
    bihn = b_ih[2 * D:].reshape(1, D)
    bhhn = b_hh[2 * D:].reshape(1, D)

    Wu, Wr, Wh = (np.asarray(inputs[k], dtype=f32) for k in ("Wu", "Wr", "Wh"))
    Uu, Ur, Uh = (np.asarray(inputs[k], dtype=f32) for k in ("Uu", "Ur", "Uh"))
    bu = np.asarray(inputs["bu"], dtype=f32).reshape(-1)
    br = np.asarray(inputs["br"], dtype=f32).reshape(-1)
    bh = np.asarray(inputs["bh"], dtype=f32).reshape(1, D)
    aux_wm = np.concatenate([Wr, Wu], axis=1)       # [D, 2D], r first
    auh_wm = np.concatenate([Ur, Uu], axis=1)
    ba = np.concatenate([br, bu]).reshape(1, 2 * D)

    W1 = np.ascontiguousarray(inputs["W1"], dtype=f32)
    b1 = np.asarray(inputs["b1"], dtype=f32).reshape(1, D)
    W2 = np.ascontiguousarray(inputs["W2"], dtype=f32)
    b2 = np.asarray(inputs["b2"], dtype=f32).reshape(1, D // 2)
    Wf = np.ascontiguousarray(inputs["Wf"], dtype=f32)
    bfv = np.asarray(inputs["bf"], dtype=f32).reshape(1, 1)
    h0 = np.asarray(inputs["h0"], dtype=f32)
    y_t_full = tl.reshape(1, B)

    cvt = lambda a: np.ascontiguousarray(a).astype(bf16)
    shared = dict(
        emb=emb, wgx=cvt(wgx_m), wgh=cvt(wgh_m), wnx=cvt(wnx_m),
        wnh=cvt(wnh_m), bg_row=cvt(bg), bihn_r=cvt(bihn), bhhn_r=cvt(bhhn),
        aux_w=cvt(aux_wm), auh_w=cvt(auh_wm), ahx_w=cvt(Wh), ahh_w=cvt(Uh),
        ba_row=cvt(ba), bh_r=cvt(bh),
        W1a=cvt(W1[0:D]), W1b=cvt(W1[D:2 * D]),
        b1=cvt(b1), W2=cvt(W2), b2=cvt(b2), Wf=cvt(Wf), bf=cvt(bfv),
        y_t=y_t_full, idx_t=tgt.reshape(B, 1).astype(np.int32))
    in_maps = []
    for c in range(NCORES):
        sl = slice(c * BL, (c + 1) * BL)
        idx_f = np.ascontiguousarray(seqs[sl].T).reshape(-1)
        idx_hc = np.ascontiguousarray(
            idx_f.reshape(NTIL, 128).T).astype(np.int32)
        y_f = np.ascontiguousarray(labs[sl, :, 0].T).reshape(-1).astype(f32)
        y_hc = np.ascontiguousarray(y_f.reshape(NTIL, 128).T)
        h0Tc = cvt(h0[sl].T)
        m = dict(shared)
        m.update(idx_h=idx_hc, y_h=y_hc, h0T=h0Tc)
        in_maps.append(m)
    return in_maps


def kernel(**inputs) -> np.ndarray:
    nc = _get_nc()
    in_maps = _prep_inputs(inputs)
    res = run_bass_kernel_spmd(nc, in_maps, core_ids=list(range(NCORES)))
    out = np.asarray(res.results[0]["out"], dtype=np.float32)
    return out.reshape(())


# revision 61
# speedup vs baseline: 1.8967x; 1.1061x over previous
"""DIEN forward-loss kernel for Trainium2, SPMD over 8 NeuronCores. V3.

Data-parallel over batch (32 rows/core), embedding replicated. Critical-path
restructure vs V2: the per-step blend h' = g*T + (1-g)*h is split into
p = g*tanh(..) and q = (1-g)*h; since W@h' = W@p + W@q, the q-side h-matmuls
for the next step issue during the current tanh window and only the p-side
matmuls remain on the serial chain. Sigmoid is split into an early r-gate
sigmoid (gates the candidate product) and a later update-gate sigmoid (only
needed post-tanh), each in its own PSUM bank so they don't falsely couple.
The x-side preactivations land in per-step PSUM regions via identity-inject
matmuls from chunk-staged SBUF buffers; candidate x-terms are staged
interleaved ([xn_G | xh_A] per step) so t2 assembly is two DVE ops. GRU and
AUGRU run LAG=8 apart (one chunk) with AUGRU x-chunks emitted in half-chunks
to meet the tighter deadline. Aux-gram and AUGRU x-matmuls read the hidden
ring directly (strided rhs), no contiguous copy. BCE uses Softplus.

PSUM banks: pgr(x2) r-gate preacts [rG|rA], pgg(x2) update-gate preacts
[gG|uA], pn(x2) candidate h-side [nhG+bhhn|nhA], ck(x2) chunk-stage/
transposes/gram/MLP.
"""
import numpy as np
import concourse.bass as bass
import concourse.bacc as bacc
import concourse.mybir as mybir
import concourse.tile as tile
from concourse.tile import add_dep_helper
from concourse.bass_utils import run_bass_kernel_spmd
from concourse.masks import make_identity

F32 = mybir.dt.float32
BF16 = mybir.dt.bfloat16
I32 = mybir.dt.int32
AF = mybir.ActivationFunctionType
OP = mybir.AluOpType

B, L, D, NV = 256, 200, 128, 500000
NCORES = 8
BL = B // NCORES          # 32 batch rows per core
NT = L * BL               # 6400 (t,b) pairs per core
NTIL = NT // 128          # 50 gather tiles
CH = 8                    # timesteps per chunk
CW = CH * BL              # 256 cols per chunk gate region
NCHUNK = L // CH          # 25
LAG = CH                  # AUGRU lags GRU by one chunk
NSLOT = L + LAG           # 208
EPS_BN = 1e-5
DICE_A = 0.1
ALPHA = 0.2
MAGIC = 0x5F3759DF
PW = BL + 4               # 36: [hA(32) | aux | pad]
DEPRI = 10 ** 6           # scheduler de-priority offset for off-chain work


def _rsqrt(nc, pool, v, out, shape, iters=3):
    """out = 1/sqrt(v) on DVE (quake seed + Newton). v >= 0."""
    p, n = shape
    iv = out.bitcast(I32)
    nc.vector.tensor_scalar(out=iv, in0=v.bitcast(I32), scalar1=1,
                            scalar2=None, op0=OP.arith_shift_right)
    nc.vector.tensor_scalar(out=iv, in0=iv, scalar1=-1, scalar2=None,
                            op0=OP.bitwise_xor)
    nc.vector.tensor_scalar(out=iv, in0=iv, scalar1=MAGIC + 1, scalar2=None,
                            op0=OP.add)
    t = pool.tile([p, n], F32, tag="rsqrt_t")
    for _ in range(iters):
        nc.vector.tensor_tensor(out=t[:], in0=v, in1=out, op=OP.mult)
        nc.vector.tensor_tensor(out=t[:], in0=t[:], in1=out, op=OP.mult)
        nc.vector.tensor_scalar(out=t[:], in0=t[:], scalar1=-0.5, scalar2=1.5,
                                op0=OP.mult, op1=OP.add)
        nc.vector.tensor_tensor(out=out, in0=out, in1=t[:], op=OP.mult)


def build_bass(upto="full"):
    nc = bacc.Bacc("TRN2", target_bir_lowering=False, num_devices=NCORES)

    # ---------------- kernel parameters ----------------
    emb = nc.declare_dram_parameter("emb", [NV, D], F32, isOutput=False)
    idx_h = nc.declare_dram_parameter("idx_h", [128, NTIL], I32, isOutput=False)
    y_h = nc.declare_dram_parameter("y_h", [128, NTIL], F32, isOutput=False)
    idx_t = nc.declare_dram_parameter("idx_t", [B, 1], I32, isOutput=False)
    # GRU weights (bf16): x-side / h-side, gate order [r | zbar]
    wgx = nc.declare_dram_parameter("wgx", [D, 2 * D], BF16, isOutput=False)
    wgh = nc.declare_dram_parameter("wgh", [D, 2 * D], BF16, isOutput=False)
    wnx = nc.declare_dram_parameter("wnx", [D, D], BF16, isOutput=False)
    wnh = nc.declare_dram_parameter("wnh", [D, D], BF16, isOutput=False)
    bg_col = nc.declare_dram_parameter("bg_col", [D, 2], F32, isOutput=False)
    bihn_c = nc.declare_dram_parameter("bihn_c", [D, 1], F32, isOutput=False)
    bhhn_r = nc.declare_dram_parameter("bhhn_r", [1, D], BF16, isOutput=False)
    # AUGRU weights (bf16): gate order [r | u]
    aux_w = nc.declare_dram_parameter("aux_w", [D, 2 * D], BF16, isOutput=False)
    auh_w = nc.declare_dram_parameter("auh_w", [D, 2 * D], BF16, isOutput=False)
    ahx_w = nc.declare_dram_parameter("ahx_w", [D, D], BF16, isOutput=False)
    ahh_w = nc.declare_dram_parameter("ahh_w", [D, D], BF16, isOutput=False)
    ba_col = nc.declare_dram_parameter("ba_col", [D, 2], F32, isOutput=False)
    bh_c = nc.declare_dram_parameter("bh_c", [D, 1], F32, isOutput=False)
    # final MLP (bf16)
    W1a = nc.declare_dram_parameter("W1a", [D, D], BF16, isOutput=False)
    W1b = nc.declare_dram_parameter("W1b", [D, D], BF16, isOutput=False)
    b1 = nc.declare_dram_parameter("b1", [1, D], BF16, isOutput=False)
    W2 = nc.declare_dram_parameter("W2", [D, D // 2], BF16, isOutput=False)
    b2 = nc.declare_dram_parameter("b2", [1, D // 2], BF16, isOutput=False)
    Wf = nc.declare_dram_parameter("Wf", [D // 2, 1], BF16, isOutput=False)
    bf = nc.declare_dram_parameter("bf", [1, 1], BF16, isOutput=False)
    h0T = nc.declare_dram_parameter("h0T", [D, BL], BF16, isOutput=False)
    y_t = nc.declare_dram_parameter("y_t", [1, B], F32, isOutput=False)
    out_p = nc.declare_dram_parameter("out", [1, 1], F32, isOutput=True)

    ploc = nc.dram_tensor("ploc", [D, PW], BF16)
    gall = nc.dram_tensor("gall", [NCORES * D, PW], BF16)

    with tile.TileContext(nc) as tc:
        with (
            tc.tile_pool(name="persist", bufs=1) as pp,
            tc.tile_pool(name="work", bufs=3) as wk,
            tc.tile_pool(name="pq", bufs=3) as pqp,
            tc.tile_pool(name="ps_pgr", bufs=2, space="PSUM") as ppgr,
            tc.tile_pool(name="ps_pgg", bufs=2, space="PSUM") as ppgg,
            tc.tile_pool(name="ps_pn", bufs=2, space="PSUM") as ppn,
            tc.tile_pool(name="ps_ck", bufs=2, space="PSUM") as pck,
        ):
            # ---------------- index loads FIRST (gathers gate the head) ----
            idx_s = pp.tile([128, NTIL], I32, tag="idx_s")
            nc.sync.dma_start(out=idx_s[:], in_=idx_h[:])
            idx_t_s = pp.tile([128, 2], I32, tag="idx_t_s")
            idx_t_d = idx_t[:].rearrange("(k p) w -> p (k w)", k=2)
            nc.sync.dma_start(out=idx_t_s[:], in_=idx_t_d)

            # constants on the gpsimd queue BEFORE the gather stream
            identf = pp.tile([128, 128], F32, tag="identf")
            make_identity(nc, identf[:])
            identb = pp.tile([128, 128], BF16, tag="identb")
            nc.vector.tensor_copy(identb[:], identf[:])
            ones_row = pp.tile([1, CW], BF16, tag="ones_row")
            nc.gpsimd.memset(ones_row[:], 1.0)
            ones_b = pp.tile([1, B], BF16, tag="ones_b")
            nc.gpsimd.memset(ones_b[:], 1.0)
            ones_col = pp.tile([128, 1], F32, tag="ones_col")
            nc.gpsimd.memset(ones_col[:], 1.0)

            X = pp.tile([128, (NSLOT + 1) * 2 * BL], BF16, tag="X")
            X_v = X[:].rearrange("p (s w) -> p s w", w=2 * BL)
            nc.gpsimd.memset(X[:, 0:2 * BL], 0.0)
            stage = pp.tile([D, PW], BF16, tag="stage")
            nc.gpsimd.memset(stage[:], 0.0)

            erows = pp.tile([128, NT], F32, tag="erows")
            itemr = pp.tile([128, 2 * D], F32, tag="itemr")
            for k in range(NTIL):
                nc.gpsimd.indirect_dma_start(
                    out=erows[:, 128 * k:128 * (k + 1)], out_offset=None,
                    in_=emb[:],
                    in_offset=bass.IndirectOffsetOnAxis(ap=idx_s[:, k:k + 1],
                                                        axis=0),
                )
            # all 256 target items gathered locally (replicated)
            for k in range(2):
                nc.gpsimd.indirect_dma_start(
                    out=itemr[:, k * D:(k + 1) * D], out_offset=None,
                    in_=emb[:],
                    in_offset=bass.IndirectOffsetOnAxis(
                        ap=idx_t_s[:, k:k + 1], axis=0),
                )

            def load(ap, shape, tag, dt=BF16):
                t = pp.tile(shape, dt, tag=tag)
                nc.sync.dma_start(out=t[:], in_=ap[:])
                return t

            wgx_s = load(wgx, [D, 2 * D], "wgx")
            wgh_s = load(wgh, [D, 2 * D], "wgh")
            wnx_s = load(wnx, [D, D], "wnx")
            wnh_s = load(wnh, [D, D], "wnh")
            bg_s = load(bg_col, [D, 2], "bg", F32)
            bihn_s = load(bihn_c, [D, 1], "bihn", F32)
            bhhn_s = load(bhhn_r, [1, D], "bhhn")
            aux_s = load(aux_w, [D, 2 * D], "aux")
            auh_s = load(auh_w, [D, 2 * D], "auh")
            ahx_s = load(ahx_w, [D, D], "ahx")
            ahh_s = load(ahh_w, [D, D], "ahh")
            ba_s = load(ba_col, [D, 2], "ba", F32)
            bh_s = load(bh_c, [D, 1], "bh", F32)
            W1a_s = load(W1a, [D, D], "W1a")
            W1b_s = load(W1b, [D, D], "W1b")
            b1_s = load(b1, [1, D], "b1")
            W2_s = load(W2, [D, D // 2], "W2")
            b2_s = load(b2, [1, D // 2], "b2")
            Wf_s = load(Wf, [D // 2, 1], "Wf")
            bf_s = load(bf, [1, 1], "bf")
            y_t_s = load(y_t, [1, B], "y_t", F32)
            y_h_s = load(y_h, [128, NTIL], "y_h", F32)
            h0s = load(h0T, [D, BL], "h0s")

            # persistent big buffers
            ET = pp.tile([128, NT], BF16, tag="ET")
            ss_all = pp.tile([128, NTIL], F32, tag="ss_all")
            sc_all = pp.tile([128, NTIL], F32, tag="sc_all")
            s_all = pp.tile([128, NTIL], F32, tag="s_all")
            # chunk staging buffers (manual double-buffer)
            gx0 = pp.tile([128, 2 * CW], BF16, tag="gx0")
            gx1 = pp.tile([128, 2 * CW], BF16, tag="gx1")
            ax0 = pp.tile([128, 2 * CW], BF16, tag="ax0")
            ax1 = pp.tile([128, 2 * CW], BF16, tag="ax1")
            # candidate x-terms, interleaved [xn|xh] per step, split per
            # half-chunk so the late xh writes never touch a tile being read
            xcA0 = pp.tile([128, CW], BF16, tag="xcA0")
            xcA1 = pp.tile([128, CW], BF16, tag="xcA1")
            xcB0 = pp.tile([128, CW], BF16, tag="xcB0")
            xcB1 = pp.tile([128, CW], BF16, tag="xcB1")
            gx, ax = [gx0, gx1], [ax0, ax1]
            xcA, xcB = [xcA0, xcA1], [xcB0, xcB1]

            # hA(-1) = h0, read by slot LAG
            nc.vector.tensor_copy(X_v[:, LAG, BL:2 * BL], h0s[:])

            # ---------- phase A: norm + transpose pipeline ----------
            def do_tile(k):
                er = erows[:, 128 * k:128 * (k + 1)]
                sq = wk.tile([128, 128], F32, tag="sq_scr")
                nc.vector.scalar_tensor_tensor(
                    out=sq[:], in0=er, scalar=0.0, in1=er,
                    op0=OP.add, op1=OP.mult, accum_out=ss_all[:, k:k + 1])
                if k % 4 == 3 or k == NTIL - 1:
                    k0 = (k // 4) * 4
                    w = k - k0 + 1
                    _rsqrt(nc, wk, ss_all[:, k0:k + 1], sc_all[:, k0:k + 1],
                           [128, w], iters=1)
                    nc.vector.tensor_scalar_min(out=sc_all[:, k0:k + 1],
                                                in0=sc_all[:, k0:k + 1],
                                                scalar1=1.0)

            def do_tile2(j):
                erj = erows[:, 128 * j:128 * (j + 1)]
                ersc = wk.tile([128, 128], BF16, tag="ersc")
                nc.vector.tensor_scalar(out=ersc[:], in0=erj,
                                        scalar1=sc_all[:, j:j + 1],
                                        scalar2=None, op0=OP.mult)
                tp = pck.tile([128, 512], F32, tag="ck")
                tpb = tp[:].bitcast(BF16)
                nc.tensor.transpose(out=tpb[:, 0:128], in_=ersc[:],
                                    identity=identb[:])
                nc.vector.tensor_copy(ET[:, 128 * j:128 * (j + 1)],
                                      tpb[:, 0:128])

            # ---------- chunk emissions ----------
            def emit_gx(c):
                # GRU x-gate preacts for chunk c -> gx[c%2]: [rx(256)|zbx(256)]
                # gate bias folded into the PSUM->SBUF cast (per-partition add)
                ecols = ET[:, c * CW:(c + 1) * CW]
                hw = CW // 2
                ck = pck.tile([128, 512], F32, tag="ck")
                for g in range(2):
                    w = wgx_s[:, g * D:(g + 1) * D]
                    for h in range(2):
                        slc = slice(g * CW + h * hw, g * CW + (h + 1) * hw)
                        esl = ecols[:, h * hw:(h + 1) * hw]
                        nc.tensor.matmul(ck[:, slc], w, esl,
                                         start=True, stop=True)
                        nc.vector.tensor_scalar(out=gx[c % 2][:, slc],
                                                in0=ck[:, slc],
                                                scalar1=bg_s[:, g:g + 1],
                                                scalar2=None, op0=OP.add)

            def emit_xn(c):
                # GRU candidate x-preacts chunk c -> xcA/xcB interleaved slots
                ecols = ET[:, c * CW:(c + 1) * CW]
                hw = CW // 2
                ck = pck.tile([128, 512], F32, tag="ck")
                for h in range(2):
                    slc = slice(h * hw, (h + 1) * hw)
                    nc.tensor.matmul(ck[:, slc], wnx_s[:],
                                     ecols[:, slc], start=True, stop=True)
                    xdst = (xcA if h == 0 else xcB)[c % 2]
                    dst = xdst[:].rearrange("p (s w) -> p s w", w=2 * BL)
                    src = ck[:, slc].rearrange("p (s w) -> p s w", w=BL)
                    nc.vector.tensor_scalar(out=dst[:, :, 0:BL],
                                            in0=src[:, :, :],
                                            scalar1=bihn_s[:, 0:1],
                                            scalar2=None, op0=OP.add)

            def emit_ah(ca, half):
                # AUGRU x-preacts for chunk ca, steps 4*half..4*half+3.
                # Gates -> ax[(ca+1)%2] ([rx|ux]), candidate -> xcA/xcB.
                t0 = ca * CH + 4 * half
                ocols = X_v[:, t0 + 1:t0 + 5, 0:BL]      # outs t0..t0+3
                hw = 4 * BL                              # 128
                ck = pck.tile([128, 512], F32, tag="ck")
                axd = ax[(ca + 1) % 2][:].rearrange("p (g w) -> p g w", g=2)
                for g in range(2):
                    slc = slice(g * hw, (g + 1) * hw)
                    nc.tensor.matmul(ck[:, slc], aux_s[:, g * D:(g + 1) * D],
                                     ocols, start=True, stop=True)
                    nc.vector.tensor_scalar(
                        out=axd[:, g, half * hw:(half + 1) * hw],
                        in0=ck[:, slc], scalar1=ba_s[:, g:g + 1],
                        scalar2=None, op0=OP.add)
                nc.tensor.matmul(ck[:, 2 * hw:3 * hw], ahx_s[:], ocols,
                                 start=True, stop=True)
                xdst = (xcA if half == 0 else xcB)[(ca + 1) % 2]
                dst = xdst[:].rearrange("p (s w) -> p s w", w=2 * BL)
                srch = ck[:, 2 * hw:3 * hw].rearrange("p (s w) -> p s w", w=BL)
                nc.vector.tensor_scalar(out=dst[:, :, BL:2 * BL],
                                        in0=srch[:, :, :],
                                        scalar1=bh_s[:, 0:1],
                                        scalar2=None, op0=OP.add)

            iT = pp.tile([128, B], BF16, tag="iT")

            def prep_item():
                # renorm all 256 target items, transpose into iT [D, B]
                for k in range(2):
                    itk = itemr[:, k * D:(k + 1) * D]
                    sqt = wk.tile([128, D], F32, tag="sqt")
                    sst = wk.tile([128, 1], F32, tag="sst")
                    nc.vector.scalar_tensor_tensor(
                        out=sqt[:], in0=itk, scalar=0.0, in1=itk,
                        op0=OP.add, op1=OP.mult, accum_out=sst[:])
                    sct = wk.tile([128, 1], F32, tag="sct")
                    _rsqrt(nc, wk, sst[:], sct[:], [128, 1], iters=2)
                    nc.vector.tensor_scalar_min(out=sct[:], in0=sct[:],
                                                scalar1=1.0)
                    itsc = wk.tile([128, D], BF16, tag="itsc")
                    nc.vector.tensor_scalar(out=itsc[:], in0=itk,
                                            scalar1=sct[:], scalar2=None,
                                            op0=OP.mult)
                    tp = pck.tile([128, 512], F32, tag="ck")
                    tpb = tp[:].bitcast(BF16)
                    nc.tensor.transpose(out=tpb[:, 0:D], in_=itsc[:],
                                        identity=identb[:])
                    nc.vector.tensor_copy(iT[:, k * D:(k + 1) * D],
                                          tpb[:, 0:D])

            aux_sig = pp.tile([128, NTIL], F32, tag="aux_sig")

            def do_aux_sig():
                # sigma(s_all) while the sigmoid table set is still loaded
                nc.scalar.activation(aux_sig[:], s_all[:], AF.Sigmoid)

            def do_aux_ln():
                # acc = ln(1-p) + y*(ln p - ln(1-p)); clamped at -100;
                # the BCE negation is folded into the final aux scale.
                p2 = pp.tile([128, NTIL], F32, tag="p2")
                nc.vector.tensor_scalar(out=p2[:], in0=aux_sig[:],
                                        scalar1=-1.0, scalar2=1.0,
                                        op0=OP.mult, op1=OP.add)
                lp = pp.tile([128, NTIL], F32, tag="lp")
                nc.scalar.activation(lp[:], aux_sig[:], AF.Ln)
                l1p = pp.tile([128, NTIL], F32, tag="l1p")
                nc.scalar.activation(l1p[:], p2[:], AF.Ln)
                nc.vector.tensor_scalar_max(out=lp[:], in0=lp[:],
                                            scalar1=-100.0)
                nc.vector.tensor_scalar_max(out=l1p[:], in0=l1p[:],
                                            scalar1=-100.0)
                nc.vector.tensor_tensor(out=lp[:], in0=lp[:], in1=l1p[:],
                                        op=OP.subtract)
                nc.vector.tensor_tensor(out=lp[:], in0=y_h_s[:], in1=lp[:],
                                        op=OP.mult)
                nc.vector.tensor_tensor(out=lp[:], in0=lp[:], in1=l1p[:],
                                        op=OP.add)
                rsum = wk.tile([128, 1], F32, tag="rsum")
                nc.vector.reduce_sum(out=rsum[:], in_=lp[:],
                                     axis=mybir.AxisListType.X)
                aps = pck.tile([128, 512], F32, tag="ck")
                nc.tensor.matmul(aps[0:1, 0:1], rsum[:], ones_col[:, 0:1],
                                 start=True, stop=True)
                return aps

            def emit_gram(blk):
                gps = pck.tile([128, 512], F32, tag="ck")
                nc.tensor.matmul(gps[:, 0:128], ET[:, 128 * blk:128 * (blk + 1)],
                                 X_v[:, 4 * blk + 1:4 * blk + 5, 0:BL],
                                 start=True, stop=True)
                gsc = wk.tile([128, 128], F32, tag="gram_scr")
                nc.vector.scalar_tensor_tensor(
                    out=gsc[:], in0=gps[:, 0:128], scalar=1.0, in1=identf[:],
                    op0=OP.mult, op1=OP.mult,
                    accum_out=s_all[:, blk:blk + 1])

            # ---------- upfront pipeline fill ----------
            for k in range(8):
                do_tile(k)
            for j in range(6):
                do_tile2(j)
            emit_gx(0)
            emit_xn(0)

            if upto == "A":
                for k in range(8, NTIL):
                    do_tile(k)
                for j in range(6, NTIL):
                    do_tile2(j)
                dbg = wk.tile([1, 1], F32, tag="res")
                nc.vector.reduce_sum(out=dbg[:], in_=ET[0:1, 0:128],
                                     axis=mybir.AxisListType.X)
                nc.sync.dma_start(out=out_p[:], in_=dbg[:])

            # ---------- recurrence slot loop ----------
            # pg layouts: pgr = [rG|rA], pgg = [gG|uA]; gp = [rG rA gG uA]
            prev_p = prev_q = None
            if upto != "A":
                for s in range(NSLOT):
                    tg, ta, o, c = s, s - LAG, s % CH, s // CH
                    gru = tg < L
                    aug = 0 <= ta < L
                    if gru and aug:
                        cs = slice(0, 2 * BL)       # pair columns
                    elif gru:
                        cs = slice(0, BL)
                    else:
                        cs = slice(BL, 2 * BL)
                    gxc, axc = gx[c % 2], ax[c % 2]
                    xcc = (xcA if o < 4 else xcB)[c % 2]
                    oh = o % 4

                    # ---- per-step PSUM bank fill ----
                    pgr = ppgr.tile([128, 2 * BL], F32, tag="pgr")
                    pgg = ppgg.tile([128, 2 * BL], F32, tag="pgg")
                    pn = ppn.tile([128, 2 * BL], F32, tag="pn")
                    if gru:
                        gxv = gxc[:].rearrange("p (g w) -> p g w", g=2)
                        # injects: r-gate then update-gate x-preacts
                        nc.tensor.matmul(pgr[:, 0:BL], identb[:],
                                         gxv[:, 0, o * BL:(o + 1) * BL],
                                         start=True, stop=(s == 0))
                        nc.tensor.matmul(pgg[:, 0:BL], identb[:],
                                         gxv[:, 1, o * BL:(o + 1) * BL],
                                         start=True, stop=(s == 0))
                        nc.tensor.matmul(pn[:, 0:BL], bhhn_s[0:1, :],
                                         ones_row[0:1, 0:BL],
                                         start=True, stop=(s == 0))
                    if aug:
                        axv = axc[:].rearrange("p (g w) -> p g w", g=2)
                        nc.tensor.matmul(pgr[:, BL:2 * BL], identb[:],
                                         axv[:, 0, o * BL:(o + 1) * BL],
                                         start=True, stop=False)
                        nc.tensor.matmul(pgg[:, BL:2 * BL], identb[:],
                                         axv[:, 1, o * BL:(o + 1) * BL],
                                         start=True, stop=False)
                    if s == LAG:
                        # AUGRU step 0: h-side terms come straight from h0
                        nc.tensor.matmul(pgr[:, BL:2 * BL], auh_s[:, 0:D],
                                         h0s[:], start=False, stop=True)
                        nc.tensor.matmul(pgg[:, BL:2 * BL], auh_s[:, D:2 * D],
                                         h0s[:], start=False, stop=True)
                        nc.tensor.matmul(pn[:, BL:2 * BL], ahh_s[:], h0s[:],
                                         start=True, stop=True)
                    # q-side h-matmuls (prev_q ready during prev tanh window)
                    if s > 0 and gru:
                        qG = prev_q[:, 0:BL]
                        nc.tensor.matmul(pgr[:, 0:BL], wgh_s[:, 0:D], qG,
                                         start=False, stop=False)
                        nc.tensor.matmul(pgg[:, 0:BL], wgh_s[:, D:2 * D], qG,
                                         start=False, stop=False)
                        nc.tensor.matmul(pn[:, 0:BL], wnh_s[:], qG,
                                         start=False, stop=False)
                    if aug and ta > 0:
                        qA = prev_q[:, BL:2 * BL]
                        nc.tensor.matmul(pgr[:, BL:2 * BL], auh_s[:, 0:D], qA,
                                         start=False, stop=False)
                        nc.tensor.matmul(pgg[:, BL:2 * BL], auh_s[:, D:2 * D],
                                         qA, start=False, stop=False)
                        nc.tensor.matmul(pn[:, BL:2 * BL], ahh_s[:], qA,
                                         start=True, stop=False)

                    # ---- off-chain work: de-prioritized, fills idle ----
                    with tc.high_priority(offset=-DEPRI):
                        if o == 1 and c + 1 < NCHUNK:
                            emit_gx(c + 1)
                        if o == 3 and c + 1 < NCHUNK:
                            emit_xn(c + 1)
                        if o == 4 and c < NCHUNK:
                            emit_ah(c, 0)
                        if o == 0 and 1 <= c <= NCHUNK:
                            emit_ah(c - 1, 1)
                        if s >= 2 and (s - 2) % 4 == 3 and (s - 2) // 4 < NTIL \
                                and s - 2 < L:
                            emit_gram((s - 2) // 4)
                        if s % 2 == 1 and 8 + (s - 1) // 2 < NTIL:
                            do_tile(8 + (s - 1) // 2)
                        if s % 2 == 1 and s >= 5 and 6 + (s - 5) // 2 < NTIL:
                            do_tile2(6 + (s - 5) // 2)
                        if s == 120:
                            prep_item()
                        if s == 204 and upto in ("X", "full"):
                            do_aux_sig()

                    # p-side h-matmuls: r-gates first (they gate sigma_r)
                    if s > 0 and gru:
                        pG = prev_p[:, 0:BL]
                        nc.tensor.matmul(pgr[:, 0:BL], wgh_s[:, 0:D], pG,
                                         start=False, stop=True)
                    if aug and ta > 0:
                        pA = prev_p[:, BL:2 * BL]
                        nc.tensor.matmul(pgr[:, BL:2 * BL], auh_s[:, 0:D], pA,
                                         start=False, stop=True)
                    if s > 0 and gru:
                        nc.tensor.matmul(pn[:, 0:BL], wnh_s[:], pG,
                                         start=False, stop=True)
                    if aug and ta > 0:
                        nc.tensor.matmul(pn[:, BL:2 * BL], ahh_s[:], pA,
                                         start=False, stop=True)
                    if s > 0 and gru:
                        nc.tensor.matmul(pgg[:, 0:BL], wgh_s[:, D:2 * D], pG,
                                         start=False, stop=True)
                    if aug and ta > 0:
                        nc.tensor.matmul(pgg[:, BL:2 * BL], auh_s[:, D:2 * D],
                                         pA, start=False, stop=True)

                    # ---- serial chain ----
                    gp = wk.tile([128, 4 * BL], BF16, tag="gp")
                    nc.scalar.activation(gp[:, cs], pgr[:, cs], AF.Sigmoid)
                    tprod = wk.tile([128, 2 * BL], BF16, tag="tprod")
                    nc.vector.tensor_tensor(out=tprod[:, cs], in0=pn[:, cs],
                                            in1=gp[:, cs], op=OP.mult)
                    t2 = wk.tile([128, 2 * BL], BF16, tag="t2")
                    t2i = nc.vector.tensor_tensor(
                        out=t2[:, cs], in0=tprod[:, cs],
                        in1=xcc[:, oh * 2 * BL + cs.start:
                                oh * 2 * BL + cs.stop],
                        op=OP.add)
                    gcs = slice(2 * BL + cs.start, 2 * BL + cs.stop)
                    nc.scalar.activation(gp[:, gcs], pgg[:, cs], AF.Sigmoid)
                    gT = wk.tile([128, 2 * BL], BF16, tag="gT")
                    nc.scalar.activation(gT[:, cs], t2[:, cs], AF.Tanh)
                    # during-tanh: zp = 1-g ; q = zp * h_prev
                    zp = wk.tile([128, 2 * BL], BF16, tag="zp")
                    zpi = nc.vector.tensor_scalar(out=zp[:, cs], in0=gp[:, gcs],
                                                  scalar1=-1.0, scalar2=1.0,
                                                  op0=OP.mult, op1=OP.add)
                    add_dep_helper(zpi.ins, t2i.ins, sync=False,
                                   reason="keep zp off the t2 chain")
                    q = pqp.tile([128, 2 * BL], BF16, tag="q")
                    nc.vector.tensor_tensor(out=q[:, cs], in0=zp[:, cs],
                                            in1=X_v[:, s, cs], op=OP.mult)
                    # post-tanh: p = g*T ; h' = p + q
                    p = pqp.tile([128, 2 * BL], BF16, tag="p")
                    nc.vector.tensor_tensor(out=p[:, cs], in0=gp[:, gcs],
                                            in1=gT[:, cs], op=OP.mult)
                    nc.vector.tensor_tensor(out=X_v[:, s + 1, cs],
                                            in0=p[:, cs], in1=q[:, cs],
                                            op=OP.add)
                    prev_p, prev_q = p, q

            if upto == "G":
                dbg = wk.tile([1, 1], F32, tag="res")
                dbf = wk.tile([1, BL], F32, tag="resb")
                nc.vector.tensor_copy(dbf[:], X_v[0:1, L, 0:BL])
                nc.vector.reduce_sum(out=dbg[:], in_=dbf[:],
                                     axis=mybir.AxisListType.X)
                nc.sync.dma_start(out=out_p[:], in_=dbg[:])
            if upto == "GA":
                dbg = wk.tile([1, 1], F32, tag="res")
                dbf = wk.tile([1, BL], F32, tag="resb")
                nc.vector.tensor_copy(dbf[:], X_v[0:1, NSLOT, BL:2 * BL])
                nc.vector.reduce_sum(out=dbg[:], in_=dbf[:],
                                     axis=mybir.AxisListType.X)
                nc.sync.dma_start(out=out_p[:], in_=dbg[:])

            if upto in ("X", "full"):
                # ---------- aux Ln + pack + AllGather (bf16) ----------
                aux_ps = do_aux_ln()
                nc.vector.tensor_copy(stage[:, 0:BL], X_v[:, NSLOT, BL:2 * BL])
                nc.vector.tensor_copy(stage[0:1, BL:BL + 1],
                                      aux_ps[0:1, 0:1])
                nc.sync.dma_start(out=ploc[:], in_=stage[:])
                nc.gpsimd.collective_compute(
                    "AllGather", OP.bypass,
                    replica_groups=[list(range(NCORES))],
                    ins=[ploc[:]], outs=[gall[:]],
                )

            if upto == "X":
                dbg = wk.tile([1, 1], F32, tag="res")
                nc.vector.tensor_copy(dbg[:], aux_ps[0:1, 0:1])
                nc.sync.dma_start(out=out_p[:], in_=dbg[:])

            if upto == "full":
                # ---------- replicated final MLP ----------
                gat = pp.tile([D, NCORES * PW], BF16, tag="gat")
                dma_engs = [nc.sync, nc.scalar, nc.gpsimd]
                for cc in range(NCORES):
                    dma_engs[cc % 3].dma_start(
                        out=gat[:, cc * PW:(cc + 1) * PW],
                        in_=gall[cc * D:(cc + 1) * D, :])
                gat_v = gat[:].rearrange("p (c w) -> p c w", c=NCORES)
                hT_v = gat_v[:, :, 0:BL]            # [128, 8, 32]
                aux8 = wk.tile([1, NCORES], F32, tag="aux8")
                aux8_v = aux8[:].rearrange("p (c w) -> p c w", w=1)
                nc.vector.tensor_copy(aux8_v, gat_v[0:1, :, BL:BL + 1])
                aux_tot = wk.tile([1, 1], F32, tag="aux_tot")
                nc.vector.reduce_sum(out=aux_tot[:], in_=aux8[:],
                                     axis=mybir.AxisListType.X)

                def dice(z_ps, pdim):
                    m = wk.tile([pdim, 1], F32, tag="dice_m")
                    nc.vector.reduce_sum(out=m[:], in_=z_ps[:],
                                         axis=mybir.AxisListType.X)
                    nc.vector.tensor_scalar_mul(out=m[:], in0=m[:],
                                                scalar1=1.0 / B)
                    xcen = wk.tile([pdim, B], F32, tag="dice_xc")
                    nc.vector.tensor_scalar(out=xcen[:], in0=z_ps[:],
                                            scalar1=m[:], scalar2=None,
                                            op0=OP.subtract)
                    sq2 = wk.tile([pdim, B], F32, tag="dice_sq")
                    vs = wk.tile([pdim, 1], F32, tag="dice_vs")
                    nc.scalar.activation(sq2[:], xcen[:], AF.Square,
                                         accum_out=vs[:])
                    nc.vector.tensor_scalar(out=vs[:], in0=vs[:],
                                            scalar1=1.0 / B, scalar2=EPS_BN,
                                            op0=OP.mult, op1=OP.add)
                    inv = wk.tile([pdim, 1], F32, tag="dice_inv")
                    _rsqrt(nc, wk, vs[:], inv[:], [pdim, 1], iters=1)
                    pr = wk.tile([pdim, B], F32, tag="dice_p")
                    nc.scalar.activation(pr[:], xcen[:], AF.Sigmoid,
                                         scale=inv[:, 0:1])
                    nc.vector.tensor_scalar(out=pr[:], in0=pr[:],
                                            scalar1=1 - DICE_A, scalar2=DICE_A,
                                            op0=OP.mult, op1=OP.add)
                    zd = wk.tile([pdim, B], BF16, tag="dice_zd")
                    nc.vector.tensor_tensor(out=zd[:], in0=z_ps[:], in1=pr[:],
                                            op=OP.mult)
                    return zd

                z1_ps = pck.tile([128, 512], F32, tag="ck")
                nc.tensor.matmul(z1_ps[:, 0:B], W1a_s[:], hT_v,
                                 start=True, stop=False)
                nc.tensor.matmul(z1_ps[:, 0:B], W1b_s[:], iT[:, :],
                                 start=False, stop=False)
                nc.tensor.matmul(z1_ps[:, 0:B], b1_s[0:1, :], ones_b[0:1, :],
                                 start=False, stop=True)
                z1d = dice(z1_ps[:, 0:B], 128)

                z2_ps = pck.tile([128, 512], F32, tag="ck")
                nc.tensor.matmul(z2_ps[0:D // 2, 0:B], W2_s[:, :], z1d[:],
                                 start=True, stop=False)
                nc.tensor.matmul(z2_ps[0:D // 2, 0:B], b2_s[0:1, :],
                                 ones_b[0:1, :], start=False, stop=True)
                z2d = dice(z2_ps[0:D // 2, 0:B], D // 2)

                s_ps = pck.tile([128, 512], F32, tag="ck")
                nc.tensor.matmul(s_ps[0:1, 0:B], Wf_s[:, 0:1], z2d[:],
                                 start=True, stop=False)
                nc.tensor.matmul(s_ps[0:1, 0:B], bf_s[0:1, 0:1],
                                 ones_b[0:1, :], start=False, stop=True)
                # final BCE in sigma/ln form (sigmoid set still loaded from
                # dice); negation folded into the final scales
                pg1 = wk.tile([1, B], F32, tag="pg1")
                nc.scalar.activation(pg1[:], s_ps[0:1, 0:B], AF.Sigmoid)
                pg2 = wk.tile([1, B], F32, tag="pg2")
                nc.vector.tensor_scalar(out=pg2[:], in0=pg1[:], scalar1=-1.0,
                                        scalar2=1.0, op0=OP.mult, op1=OP.add)
                lq1 = wk.tile([1, B], F32, tag="lq1")
                nc.scalar.activation(lq1[:], pg1[:], AF.Ln)
                lq2 = wk.tile([1, B], F32, tag="lq2")
                nc.scalar.activation(lq2[:], pg2[:], AF.Ln)
                nc.vector.tensor_scalar_max(out=lq1[:], in0=lq1[:],
                                            scalar1=-100.0)
                nc.vector.tensor_scalar_max(out=lq2[:], in0=lq2[:],
                                            scalar1=-100.0)
                nc.vector.tensor_tensor(out=lq1[:], in0=lq1[:], in1=lq2[:],
                                        op=OP.subtract)
                nc.vector.tensor_tensor(out=lq1[:], in0=y_t_s[:], in1=lq1[:],
                                        op=OP.mult)
                nc.vector.tensor_tensor(out=lq1[:], in0=lq1[:], in1=lq2[:],
                                        op=OP.add)
                rec_sum = wk.tile([1, 1], F32, tag="rec_sum")
                nc.vector.reduce_sum(out=rec_sum[:], in_=lq1[:],
                                     axis=mybir.AxisListType.X)

                nc.vector.tensor_scalar_mul(out=aux_tot[:], in0=aux_tot[:],
                                            scalar1=-ALPHA / (B * L))
                nc.vector.tensor_scalar_mul(out=rec_sum[:], in0=rec_sum[:],
                                            scalar1=-1.0 / B)
                res = wk.tile([1, 1], F32, tag="res")
                nc.vector.tensor_tensor(out=res[:], in0=aux_tot[:],
                                        in1=rec_sum[:], op=OP.add)
                nc.sync.dma_start(out=out_p[:], in_=res[:])
    nc.compile()
    return nc


_NC_CACHE = None


def _get_nc():
    global _NC_CACHE
    if _NC_CACHE is None:
        import os
        _NC_CACHE = build_bass(os.environ.get("KERNEL_UPTO", "full"))
    return _NC_CACHE


def _prep_inputs(inputs):
    f32 = np.float32
    import ml_dtypes
    bf16 = ml_dtypes.bfloat16
    emb = np.ascontiguousarray(inputs["emb"], dtype=f32)
    seqs = np.asarray(inputs["history_seqs"])
    labs = np.asarray(inputs["history_labels"])
    tgt = np.asarray(inputs["target_item"])
    tl = np.asarray(inputs["target_label"]).astype(f32)

    w_ih = np.asarray(inputs["w_ih"], dtype=f32)   # rows: [r | z | n]
    w_hh = np.asarray(inputs["w_hh"], dtype=f32)
    b_ih = np.asarray(inputs["b_ih"], dtype=f32)
    b_hh = np.asarray(inputs["b_hh"], dtype=f32)
    # gate order in banks: [r | zbar]; zbar = negated z
    wgx_m = np.concatenate([w_ih[0:D].T, -w_ih[D:2 * D].T], axis=1)
    wgh_m = np.concatenate([w_hh[0:D].T, -w_hh[D:2 * D].T], axis=1)
    bg = np.stack([b_ih[0:D] + b_hh[0:D],
                   -(b_ih[D:2 * D] + b_hh[D:2 * D])], axis=1)  # [D, 2]
    wnx_m = np.ascontiguousarray(w_ih[2 * D:3 * D].T)
    wnh_m = np.ascontiguousarray(w_hh[2 * D:3 * D].T)
    bihn = b_ih[2 * D:].reshape(D, 1)
    bhhn = b_hh[2 * D:].reshape(1, D)

    Wu, Wr, Wh = (np.asarray(inputs[k], dtype=f32) for k in ("Wu", "Wr", "Wh"))
    Uu, Ur, Uh = (np.asarray(inputs[k], dtype=f32) for k in ("Uu", "Ur", "Uh"))
    bu = np.asarray(inputs["bu"], dtype=f32).reshape(-1)
    br = np.asarray(inputs["br"], dtype=f32).reshape(-1)
    bh = np.asarray(inputs["bh"], dtype=f32).reshape(D, 1)
    aux_wm = np.concatenate([Wr, Wu], axis=1)       # [D, 2D], r first
    auh_wm = np.concatenate([Ur, Uu], axis=1)
    ba = np.stack([br, bu], axis=1)                 # [D, 2]

    W1 = np.ascontiguousarray(inputs["W1"], dtype=f32)
    b1 = np.asarray(inputs["b1"], dtype=f32).reshape(1, D)
    W2 = np.ascontiguousarray(inputs["W2"], dtype=f32)
    b2 = np.asarray(inputs["b2"], dtype=f32).reshape(1, D // 2)
    Wf = np.ascontiguousarray(inputs["Wf"], dtype=f32)
    bfv = np.asarray(inputs["bf"], dtype=f32).reshape(1, 1)
    h0 = np.asarray(inputs["h0"], dtype=f32)
    y_t_full = tl.reshape(1, B)

    cvt = lambda a: np.ascontiguousarray(a).astype(bf16)
    c32 = lambda a: np.ascontiguousarray(a, dtype=f32)
    shared = dict(
        emb=emb, wgx=cvt(wgx_m), wgh=cvt(wgh_m), wnx=cvt(wnx_m),
        wnh=cvt(wnh_m), bg_col=c32(bg), bihn_c=c32(bihn), bhhn_r=cvt(bhhn),
        aux_w=cvt(aux_wm), auh_w=cvt(auh_wm), ahx_w=cvt(Wh), ahh_w=cvt(Uh),
        ba_col=c32(ba), bh_c=c32(bh),
        W1a=cvt(W1[0:D]), W1b=cvt(W1[D:2 * D]),
        b1=cvt(b1), W2=cvt(W2), b2=cvt(b2), Wf=cvt(Wf), bf=cvt(bfv),
        y_t=y_t_full, idx_t=tgt.reshape(B, 1).astype(np.int32))
    in_maps = []
    for c in range(NCORES):
        sl = slice(c * BL, (c + 1) * BL)
        idx_f = np.ascontiguousarray(seqs[sl].T).reshape(-1)
        idx_hc = np.ascontiguousarray(
            idx_f.reshape(NTIL, 128).T).astype(np.int32)
        y_f = np.ascontiguousarray(labs[sl, :, 0].T).reshape(-1).astype(f32)
        y_hc = np.ascontiguousarray(y_f.reshape(NTIL, 128).T)
        h0Tc = cvt(h0[sl].T)
        m = dict(shared)
        m.update(idx_h=idx_hc, y_h=y_hc, h0T=h0Tc)
        in_maps.append(m)
    return in_maps


def kernel(**inputs) -> np.ndarray:
    nc = _get_nc()
    in_maps = _prep_inputs(inputs)
    res = run_bass_kernel_spmd(nc, in_maps, core_ids=list(range(NCORES)))
    out = np.asarray(res.results[0]["out"], dtype=np.float32)
    return out.reshape(())
